# revision 56
# baseline (speedup 1.0000x reference)
"""Differentiable particle filter V3 — Trainium2 Bass kernel (optimized).

Data-parallel over batch B=16 across 8 NeuronCores (2 batch items/core).
Each core runs the T=16 sequential scan for its two particle clouds
(N=512, dL=128) with activations on-chip, feature-on-partition /
particle-on-free layout so MLP layers are PE matmul chains.

Optimizations vs v1:
- Host precomputes: per-(t,b) layer-1 biases (h contraction + b3m fold),
  y' = y - oe_b3[0], 0.5*sum(eps^2)+const, t=0 regime softmax, and
  V = emb @ W_e folding (kills remb matmuls and all on-device h work).
- 2 ACT table-set switches/step: exp(log_std) via tanh identity
  (tanh lives in the silu table set), whole tail uses natural_log_exp set.
- s_j folded into resample lhs (znew_s, rl9s); denominator rides g2 as a
  9th lhs column; 1/D broadcast via K=1 PE matmul (no DRAM bounce).
- Cross-partition max/sum via gpsimd.partition_all_reduce.
- 3 DMA descriptors per step issued from the idle sync engine.
- Resample skipped on the final step (carry unused).
"""

import numpy as np
import ml_dtypes

import concourse.bass as bass
import concourse.tile as tile
from concourse import mybir
from concourse import bass_isa
from concourse.masks import make_identity
from concourse.bass import ts

F32 = mybir.dt.float32
F32R = mybir.dt.float32r
F16 = mybir.dt.float16
BF16 = mybir.dt.bfloat16
AF = mybir.ActivationFunctionType
ALU = mybir.AluOpType
AX = mybir.AxisListType
RED = bass_isa.ReduceOp

LOG2PI = 1.8378770664093453

# problem dims (hardcoded per spec)
B, N, T_FULL = 16, 512, 16
dL, dM, dE, H = 128, 256, 32, 256
Kt, Ka = 18, 8
NCORES = 8
B2 = 2          # batch items per core
NC = 4          # 128-particle chunks per batch item
NCB = NC * B2   # particle chunks per core (8)
NT = B2 * N     # particles per core (1024)
NHB = 22        # host bias pack columns: 12 l1-bias, 2 y', 8 qeps


def split_waits(nc, limit=1):
    """This walrus build encodes at most one sync wait per instruction.
    Hoist excess waits onto injected same-engine NOPs placed immediately
    before the instruction (engine program order preserves semantics)."""
    for f in nc.m.functions:
        for bb in f.blocks:
            newl = []
            for ins in bb.instructions:
                si = ins.sync_info
                if si is not None and si.on_wait and len(si.on_wait) > limit:
                    waits = list(si.on_wait)
                    for k, wv in enumerate(waits[:-limit]):
                        nop = mybir.InstNoOp(
                            name=f"{ins.name}-ws{k}", ins=[], outs=[])
                        nop.engine = ins.engine
                        nop.sync_info = mybir.SyncInfo(on_wait=[wv], on_update=[])
                        newl.append(nop)
                    si.on_wait = waits[-limit:]
                newl.append(ins)
            try:
                bb.instructions = newl
            except Exception:
                bb.instructions.clear()
                bb.instructions.extend(newl)
    return nc


def build_core_program(t_steps=T_FULL):
    nc = bass.Bass()

    # ---------------- DRAM parameters (per-core shapes) ----------------
    d_pT = nc.declare_dram_parameter("pT", [t_steps, B2, N, N], BF16, isOutput=False)
    d_eps = nc.declare_dram_parameter("eps_n", [t_steps, B2, N, dL], F16, isOutput=False)
    d_hb = nc.declare_dram_parameter("hb", [t_steps, 128, NHB], F32, isOutput=False)
    d_z0T = nc.declare_dram_parameter("z0T", [dL, NT], F16, isOutput=False)
    d_rp0 = nc.declare_dram_parameter("rp0T", [Ka, NT], F16, isOutput=False)

    d_w = {}
    for name, shape, dt in [
        ("pz1_z", [dL, H], F16), ("pr1_z", [dL, H], F16), ("oe1_z", [dL, H], F16),
        ("vpz", [Ka, H], F16), ("vpr", [Ka, H], F16),
        ("pz2", [H, H], F16), ("pz3", [H, 2 * dL], F16),
        ("pr2", [H, Ka], F16),
        ("oe2", [H, 128], F16), ("oe3", [128, 2], F16),
        ("b2z", [128, 2], F32),          # pz_b2 cols
        ("b2o", [128, 1], F32),          # oe_b2 col
        ("b3s_rep", [128, NC * dL], F32),  # pz_b3[dL:] row replicated+tiled 4x
        ("b3m_col", [128, 1], F32),      # pz_b3[:dL] as partition col
        ("oe3b1", [128, 1], F32),        # oe_b3[1] replicated col
        ("scales_rep", [128, NCB * Ka], F32),  # softplus(scales) tiled 8x
        ("prb2_rep", [128, NCB * Ka], F32),    # pr_b2 tiled 8x
    ]:
        d_w[name] = nc.declare_dram_parameter(name, shape, dt, isOutput=False)

    d_out = nc.declare_dram_parameter("means", [B2, dL, t_steps], F32, isOutput=True)

    from contextlib import ExitStack
    with tile.TileContext(nc) as tc, ExitStack() as ctx:
        wp = ctx.enter_context(tc.tile_pool(name="wp", bufs=1))
        sp = ctx.enter_context(tc.tile_pool(name="sp", bufs=2))
        pp = ctx.enter_context(tc.tile_pool(name="pp", bufs=2))
        st = ctx.enter_context(tc.tile_pool(name="st", bufs=1))
        psA = ctx.enter_context(tc.tile_pool(name="psA", bufs=5, space="PSUM"))
        psC = ctx.enter_context(tc.tile_pool(name="psC", bufs=3, space="PSUM"))

        nV, nS, nG, nTe, nY = nc.vector, nc.scalar, nc.gpsimd, nc.tensor, nc.sync

        # ---------------- load weights / constants ----------------
        w = {}
        for wi, (name, h) in enumerate(d_w.items()):
            eng = nG if wi % 2 == 0 else nY
            shp = list(h.shape)
            if shp[0] > 128:
                assert shp[0] == 256
                tl = wp.tile([128, 2, shp[1]], h.dtype, name="w_" + name)
                eng.dma_start(out=tl, in_=h[:, :].rearrange("(c p) m -> p c m", p=128))
            else:
                tl = wp.tile(shp, h.dtype, name="w_" + name)
                eng.dma_start(out=tl, in_=h[:, :])
            w[name] = tl

        ident16 = wp.tile([128, 128], F16, name="ident16")
        make_identity(nc, ident16)
        ident32 = wp.tile([128, 128], F32, name="ident32")
        make_identity(nc, ident32)
        ones16 = wp.tile([1, 128], F16, name="ones16")
        nV.memset(ones16, 1.0)
        ones32 = wp.tile([1, 128], F32, name="ones32")
        nV.memset(ones32, 1.0)
        ones_bf = wp.tile([1, 128], BF16, name="ones_bf")
        nV.memset(ones_bf, 1.0)
        ones_r = wp.tile([1, 128], F32R, name="ones_r")
        nV.tensor_copy(out=ones_r, in_=ones32)


        # ---------------- state ----------------
        zT = st.tile([dL, NT], F16, name="zT_state")
        nG.dma_start(out=zT, in_=d_z0T[:, :])
        rp0 = st.tile([Ka, NT], F16, name="rp0_state")
        nG.dma_start(out=rp0, in_=d_rp0[:, :])
        # row 0 = D/D = 1 (ignored), rows 1..8 = normalized regime logits
        rlr_rows = st.tile([Ka + 1, NT], F32, name="rlr_rows")
        means_acc = st.tile([dL, B2, t_steps], F32, name="means_acc")

        # Collapse the weight/state-load DMA deps into one barrier (see
        # split_waits): a chain of tiny DVE reads accumulates every DMA
        # tick into DVE's vector clock; one NOP then covers all loads.
        from concourse.tile import add_dep_helper
        probe = st.tile([1, 1], F32, name="probe")
        last_copy = None
        for tl in [*w.values(), ident16, zT, rp0]:
            src = tl[0:1, 0, 0:1] if len(tl.shape) == 3 else tl[0:1, 0:1]
            last_copy = nV.tensor_copy(out=probe, in_=src)
        curr_bb = nc.cur_bb
        bar = nc.sync.nop()
        assert last_copy is not None
        add_dep_helper(bar.ins, last_copy.ins, sync=True, reason="weights barrier")
        tc.barrier_instruction_and_bb = (bar.ins, curr_bb)

        for t in range(t_steps):
            last = (t == t_steps - 1)
            # ---------------- step input DMAs (sync engine) ----------------
            if not last:
                P_t = pp.tile([128, NCB, N], BF16, name="P_t")
                nY.dma_start(out=P_t,
                             in_=d_pT[t].rearrange("b (c p) i -> p (b c) i", p=128))
            eps_t = sp.tile([128, NCB, dL], F16, name="eps_t")
            nY.dma_start(out=eps_t,
                         in_=d_eps[t].rearrange("b (c p) d -> p (b c) d", p=128))
            hb_t = sp.tile([128, NHB], F32, name="hb_t")
            nY.dma_start(out=hb_t, in_=d_hb[t])

            def l1bias(net, m, b):
                c = net * 4 + m * 2 + b
                return hb_t[:, c:c + 1]

            # pz layer-1 z-passes first: PE chews on these (psum left open,
            # start w/o stop) while the rl-softmax block runs on DVE/ACT.
            pz_zps = []
            for m in range(2):
                row = []
                for b in range(B2):
                    ps1 = psA.tile([128, N], F32, tag="ps", name="l1")
                    nTe.matmul(ps1, w["pz1_z"][:, ts(m, 128)], zT[:, ts(b, N)],
                               start=True, stop=False)
                    row.append(ps1)
                pz_zps.append(row)

            # ---------------- regime softmax -> rp_rows (8, NT) f16 -------
            # ACT exp here rides the ln_exp table set left over from the
            # previous step's tail (no switch).
            if t == 0:
                rp_rows = rp0
            else:
                rp_rows = sp.tile([Ka, NT], F16, name="rp_rows")
                rlc = sp.tile([128, NCB, Ka + 1], F32, name="rlc")
                for c in range(NCB):
                    tp = psC.tile([128, Ka + 1], F32, tag="ps", name="rltp")
                    nTe.transpose(tp, rlr_rows[:, ts(c, 128)],
                                  ident32[0:Ka + 1, 0:Ka + 1])
                    if c % 2 == 0:
                        nV.tensor_copy(out=rlc[:, c, :], in_=tp)
                    else:
                        nS.activation(out=rlc[:, c, :], in_=tp, func=AF.Copy)
                ernr = sp.tile([128, NCB, Ka], F32, name="ernr")
                nS.activation(out=ernr, in_=rlc[:, :, 1:Ka + 1], func=AF.Exp)
                dnr = sp.tile([128, NCB], F32, name="dnr")
                nV.tensor_reduce(out=dnr, in_=ernr, axis=AX.X, op=ALU.add)
                nV.tensor_scalar_add(dnr, dnr, float(Kt - Ka))
                rdnr = sp.tile([128, NCB], F32, name="rdnr")
                nV.reciprocal(out=rdnr, in_=dnr)
                rpc = sp.tile([128, NCB, Ka], F16, name="rpc")
                for c in range(NCB):
                    nV.tensor_scalar_mul(rpc[:, c, :], ernr[:, c, :],
                                         rdnr[:, c:c + 1])
                for c in range(NCB):
                    tpb = psC.tile([Ka, 128], F16, tag="ps", name="rptp")
                    nTe.transpose(tpb, rpc[:, c, :], ident16)
                    if c % 2 == 0:
                        nV.tensor_copy(out=rp_rows[:, ts(c, 128)], in_=tpb)
                    else:
                        nS.activation(out=rp_rows[:, ts(c, 128)], in_=tpb,
                                      func=AF.Copy)

            # ---------------- layer 1 (pz, pr) ----------------
            # z-passes for pz were already issued before the rl-softmax
            # block (PE overlap); close them with the e-pass then silu.
            def layer1(wz, ve, net, nm, zps=None):
                out = []
                for m in range(2):
                    ht = sp.tile([128, NT], F16, name=nm + str(m))
                    for b in range(B2):
                        if zps is None:
                            ps1 = psA.tile([128, N], F32, tag="ps", name="l1")
                            nTe.matmul(ps1, wz[:, ts(m, 128)], zT[:, ts(b, N)],
                                       start=True, stop=False)
                        else:
                            ps1 = zps[m][b]
                        nTe.matmul(ps1, ve[:, ts(m, 128)], rp_rows[:, ts(b, N)],
                                   start=False, stop=True)
                        nS.activation(out=ht[:, ts(b, N)], in_=ps1, func=AF.Silu,
                                      bias=l1bias(net, m, b))
                    out.append(ht)
                return out

            hz1 = layer1(w["pz1_z"], w["vpz"], 0, "hz1_", zps=pz_zps)
            prh = layer1(w["pr1_z"], w["vpr"], 1, "prh_")

            # ---------------- layer 2 (pz2) ----------------
            hz2 = []
            for m in range(2):
                ht = sp.tile([128, NT], F16, name="hz2_" + str(m))
                for b in range(B2):
                    ps2 = psA.tile([128, N], F32, tag="ps", name="l2")
                    nTe.matmul(ps2, w["pz2"][:, 0, ts(m, 128)], hz1[0][:, ts(b, N)],
                               start=True, stop=False)
                    nTe.matmul(ps2, w["pz2"][:, 1, ts(m, 128)], hz1[1][:, ts(b, N)],
                               start=False, stop=True)
                    nS.activation(out=ht[:, ts(b, N)], in_=ps2, func=AF.Silu,
                                  bias=w["b2z"][:, m:m + 1])
                hz2.append(ht)

            # ---------------- pz3 (flip) + znew + log-q pieces -------------
            znew16 = sp.tile([128, NCB, dL], F16, name="znew16")
            qls = sp.tile([128, NCB], F32, name="qls")
            NH = 2  # chunks per pz3 psum tile
            for b in range(B2):
                for h2 in range(NC // NH):
                    c0 = b * NC + h2 * NH
                    zps = psA.tile([128, NH, 2 * dL], F32, tag="ps", name="zp")
                    for jj in range(NH):
                        nTe.matmul(zps[:, jj, :], hz2[0][:, ts(c0 + jj, 128)],
                                   w["pz3"][:, 0, :], start=True, stop=False)
                        nTe.matmul(zps[:, jj, :], hz2[1][:, ts(c0 + jj, 128)],
                                   w["pz3"][:, 1, :], start=False, stop=True)
                    # ls = clip(raw + b3s, -5, 2); qls = sum_d ls
                    lsa = sp.tile([128, NH, dL], F32, name="lsa")
                    nV.tensor_tensor(out=lsa, in0=zps[:, :, dL:2 * dL],
                                     in1=w["b3s_rep"][:, 0:NH * dL].rearrange(
                                         "p (c d) -> p c d", c=NH),
                                     op=ALU.add)
                    nV.tensor_scalar(lsa, lsa, 2.0, -5.0,
                                     op0=ALU.min, op1=ALU.max)
                    nV.tensor_reduce(out=qls[:, c0:c0 + NH], in_=lsa,
                                     axis=AX.X, op=ALU.add)
                    els = sp.tile([128, NH, dL], F32, name="els")
                    nS.activation(out=els, in_=lsa, func=AF.Exp)
                    p1 = sp.tile([128, NH, dL], F32, name="p1")
                    nV.tensor_mul(p1, els, eps_t[:, c0:c0 + NH, :])
                    nV.tensor_tensor(out=znew16[:, c0:c0 + NH, :], in0=p1,
                                     in1=zps[:, :, 0:dL], op=ALU.add)

            # ---------------- pr layer-2 (flip) + regime mix --------------
            # emitted here: pr2 matmuls give PE fill during the znew chain,
            # and ern's exp joins the els exp group (no extra table switch).
            r2ps = psC.tile([128, NCB, Ka], F32, tag="ps", name="pr2")
            for c in range(NCB):
                nTe.matmul(r2ps[:, c, :], prh[0][:, ts(c, 128)], w["pr2"][:, 0, :],
                           start=True, stop=False)
                nTe.matmul(r2ps[:, c, :], prh[1][:, ts(c, 128)], w["pr2"][:, 1, :],
                           start=False, stop=True)
            rlog = sp.tile([128, NCB, Ka], F32, name="rlog")
            nV.tensor_tensor(out=rlog, in0=r2ps,
                             in1=w["prb2_rep"][:, :].rearrange(
                                 "p (c k) -> p c k", c=NCB), op=ALU.add)
            ern = sp.tile([128, NCB, Ka], F32, name="ern")
            nS.activation(out=ern, in_=rlog, func=AF.Exp)
            dn = sp.tile([128, NCB], F32, name="dn")
            nV.tensor_reduce(out=dn, in_=ern, axis=AX.X, op=ALU.add)
            nV.tensor_scalar_add(dn, dn, float(Kt - Ka))
            rdn = sp.tile([128, NCB], F32, name="rdn")
            nV.reciprocal(out=rdn, in_=dn)
            smu = sp.tile([128, NCB, Ka], F32, name="smu")
            nV.tensor_tensor(out=smu, in0=ern,
                             in1=w["scales_rep"][:, :].rearrange(
                                 "p (c k) -> p c k", c=NCB), op=ALU.mult)
            smult = sp.tile([128, NCB], F32, name="smult")
            nV.tensor_reduce(out=smult, in_=smu, axis=AX.X, op=ALU.add)
            nV.tensor_mul(smult, smult, rdn)

            # ---------------- znT transpose ----------------
            znT = sp.tile([dL, NT], F16, name="znT")
            for c in range(NCB):
                tps = psC.tile([128, 128], F16, tag="ps", name="ztp")
                nTe.transpose(tps, znew16[:, c, :], ident16)
                if c % 2 == 0:
                    nV.tensor_copy(out=znT[:, ts(c, 128)], in_=tps)
                else:
                    nS.activation(out=znT[:, ts(c, 128)], in_=tps, func=AF.Copy)

            # ---------------- observation net ----------------
            oeh = []
            for m in range(2):
                ht = sp.tile([128, NT], F16, name="oeh_" + str(m))
                for b in range(B2):
                    pso = psA.tile([128, N], F32, tag="ps", name="o1")
                    nTe.matmul(pso, w["oe1_z"][:, ts(m, 128)], znT[:, ts(b, N)],
                               start=True, stop=True)
                    nS.activation(out=ht[:, ts(b, N)], in_=pso, func=AF.Silu,
                                  bias=l1bias(2, m, b))
                oeh.append(ht)
            em2 = sp.tile([128, NT], F16, name="em2")
            for b in range(B2):
                pso = psA.tile([128, N], F32, tag="ps", name="o2")
                nTe.matmul(pso, w["oe2"][:, 0, :], oeh[0][:, ts(b, N)],
                           start=True, stop=False)
                nTe.matmul(pso, w["oe2"][:, 1, :], oeh[1][:, ts(b, N)],
                           start=False, stop=True)
                nS.activation(out=em2[:, ts(b, N)], in_=pso, func=AF.Silu,
                              bias=w["b2o"][:, 0:1])
            o3ps = psC.tile([128, NCB, 2], F32, tag="ps", name="o3")
            for c in range(NCB):
                nTe.matmul(o3ps[:, c, :], em2[:, ts(c, 128)], w["oe3"],
                           start=True, stop=True)

            # ======= tail (natural_log_exp table set from here) =======
            u_t = sp.tile([128, NCB], F32, name="u_t")
            nS.activation(out=u_t, in_=o3ps[:, :, 1], func=AF.Exp,
                          bias=w["oe3b1"][:, 0:1])
            spv = sp.tile([128, NCB], F32, name="spv")
            nS.activation(out=spv, in_=u_t, func=AF.Ln, bias=1.0)  # softplus

            sig = sp.tile([128, NCB], F32, name="sig")
            nV.tensor_mul(sig, spv, smult)
            nV.tensor_scalar(sig, sig, 5.0, 0.1, op0=ALU.min, op1=ALU.max)
            rsig = sp.tile([128, NCB], F32, name="rsig")
            nV.reciprocal(out=rsig, in_=sig)
            dev = sp.tile([128, NCB], F32, name="dev")
            for b in range(B2):
                nV.tensor_scalar_sub(dev[:, b * NC:(b + 1) * NC],
                                     o3ps[:, b * NC:(b + 1) * NC, 0],
                                     hb_t[:, 12 + b:13 + b])
            nV.tensor_mul(dev, dev, rsig)
            sq = sp.tile([128, NCB], F32, name="sq")
            nV.tensor_mul(sq, dev, dev)
            qtot = sp.tile([128, NCB], F32, name="qtot")
            nV.tensor_tensor(out=qtot, in0=qls, in1=hb_t[:, 14:22], op=ALU.add)
            # lw = lwn - ln(sig): keep the ln implicit (exp(-ln sig) = rsig).
            # M' = max(lwn) + ln(5) >= max(lw) since -ln(sig) <= ln(1/0.1).
            lwn = sp.tile([128, NCB], F32, name="lwn")
            nV.scalar_tensor_tensor(out=lwn, in0=sq, scalar=-0.5, in1=qtot,
                                    op0=ALU.mult, op1=ALU.add)

            mxc = sp.tile([128, B2], F32, name="mxc")
            for b in range(B2):
                nV.tensor_reduce(out=mxc[:, b:b + 1],
                                 in_=lwn[:, b * NC:(b + 1) * NC],
                                 axis=AX.X, op=ALU.max)
            mrow = sp.tile([1, B2], F32, name="mrow")
            nG.tensor_reduce(out=mrow, in_=mxc, axis=AX.C, op=ALU.max)
            nG.tensor_scalar(mrow, mrow, -1.0, -2.302586, op0=ALU.mult,
                             op1=ALU.add)
            nmb_ps = psC.tile([128, B2], F32, tag="ps", name="nmb_ps")
            nTe.matmul(nmb_ps, ones32, mrow, start=True, stop=True)
            nmb = sp.tile([128, B2], F32, name="nmb")
            nV.tensor_copy(out=nmb, in_=nmb_ps)
            lwm = sp.tile([128, NCB], F32, name="lwm")
            for b in range(B2):
                nV.tensor_scalar_add(lwm[:, b * NC:(b + 1) * NC],
                                     lwn[:, b * NC:(b + 1) * NC],
                                     nmb[:, b:b + 1])
            ewp = sp.tile([128, NCB], F32, name="ewp")
            nS.activation(out=ewp, in_=lwm, func=AF.Exp)
            e_w = sp.tile([128, NCB], F32, name="e_w")
            nV.tensor_mul(e_w, ewp, rsig)
            ew16 = sp.tile([128, NCB], F16, name="ew16")
            nV.tensor_copy(out=ew16, in_=e_w)

            # weighted-mean output
            sw = sp.tile([128, B2], F32, name="sw")
            for b in range(B2):
                nV.tensor_reduce(out=sw[:, b:b + 1],
                                 in_=e_w[:, b * NC:(b + 1) * NC],
                                 axis=AX.X, op=ALU.add)
            swrow = sp.tile([1, B2], F32, name="swrow")
            nG.tensor_reduce(out=swrow, in_=sw, axis=AX.C, op=ALU.add)
            swa_ps = psC.tile([128, B2], F32, tag="ps", name="swa_ps")
            nTe.matmul(swa_ps, ones32, swrow, start=True, stop=True)
            rse = sp.tile([128, B2], F32, name="rse")
            nV.reciprocal(out=rse, in_=swa_ps)
            for b in range(B2):
                mz = psC.tile([128, 1], F32, tag="ps", name="mz")
                for jc in range(NC):
                    c = b * NC + jc
                    nTe.matmul(mz, znew16[:, c, :], ew16[:, c:c + 1],
                               start=(jc == 0), stop=(jc == NC - 1))
                nV.scalar_tensor_tensor(out=means_acc[:, b, t:t + 1], in0=mz,
                                        scalar=rse[:, b:b + 1],
                                        in1=w["b3m_col"],
                                        op0=ALU.mult, op1=ALU.add)

            # ---------------- soft resample (skipped on last step) --------
            if last:
                continue
            sj = sp.tile([128, NCB], F32, name="sj")
            nV.tensor_mul(sj, e_w, e_w)  # exp(2*(lw-M))
            zs = sp.tile([128, NCB, dL], BF16, name="zs")
            rl9s = sp.tile([128, NCB, Ka + 1], BF16, name="rl9s")
            nV.tensor_copy(out=rl9s[:, :, 0], in_=sj)
            g1s, g2s = [], []
            for b in range(B2):
                # per-b prep on gpsimd (sbuf-only) so PE can start b0's
                # matmuls while b1's prep still runs
                for jc in range(NC):
                    c = b * NC + jc
                    nV.tensor_scalar_mul(zs[:, c, :], znew16[:, c, :],
                                         sj[:, c:c + 1])
                    nV.tensor_scalar_mul(rl9s[:, c, 1:Ka + 1], rlog[:, c, :],
                                         sj[:, c:c + 1])
                g1 = psA.tile([dL, N], F32, tag="ps", name="g1")
                g2 = psA.tile([Ka + 1, N], F32, tag="ps", name="g2")
                for jc in range(NC):
                    c = b * NC + jc
                    nTe.matmul(g1, zs[:, c, :], P_t[:, c, :],
                               start=(jc == 0), stop=(jc == NC - 1))
                    nTe.matmul(g2, rl9s[:, c, :], P_t[:, c, :],
                               start=(jc == 0), stop=(jc == NC - 1))
                g1s.append(g1)
                g2s.append(g2)
                # 1/D row via Ln -> Exp(-1) on ACT (ln_exp set, no switch);
                # f32r PE broadcast (1 cyc/row), one psum->sbuf copy.
                dln = sp.tile([1, N], F32, name="dln")
                nS.activation(out=dln, in_=g2[0:1, :], func=AF.Ln)
                rdr = sp.tile([1, N], F32R, name="rdr")
                nS.activation(out=rdr, in_=dln, func=AF.Exp, scale=-1.0)
                rdbc = psA.tile([128, N], F32, tag="ps", name="rdbc")
                nTe.matmul(rdbc, ones_r, rdr, start=True, stop=True)
                rdsb = sp.tile([128, N], F32, name="rdsb")
                nV.tensor_copy(out=rdsb, in_=rdbc)
                nV.tensor_mul(zT[:, ts(b, N)], g1, rdsb)
                nV.tensor_mul(rlr_rows[:, ts(b, N)], g2,
                              rdsb[0:Ka + 1, :])

        # ---------------- write outputs ----------------
        for b in range(B2):
            nY.dma_start(out=d_out[b], in_=means_acc[:, b, :])

    return split_waits(nc)


# ======================= host side =======================

def _f16(x):
    return np.asarray(x, np.float32).astype(np.float16)


def _bf16(x):
    return np.asarray(x, np.float32).astype(ml_dtypes.bfloat16)


def _rep_row(row, p=128):
    """replicate a row vector across 128 partitions."""
    r = np.asarray(row, np.float32).reshape(-1)
    return np.broadcast_to(r[None, :], (p, r.shape[0])).copy()


def host_prep(inputs, t_steps=T_FULL):
    obs = np.asarray(inputs["obs"], np.float32)[:t_steps]
    h_seq = np.asarray(inputs["h_seq"], np.float32)[:t_steps]
    z0 = np.asarray(inputs["z0"], np.float32)
    rl0 = np.asarray(inputs["regime_logits0"], np.float32)
    eps = np.asarray(inputs["eps"], np.float32)[:t_steps]
    u = np.asarray(inputs["gumbel_u"], np.float32)[:t_steps]
    assert int(inputs["k_active"]) == Ka

    pz_w1 = np.asarray(inputs["pz_w1"], np.float32)
    pr_w1 = np.asarray(inputs["pr_w1"], np.float32)
    oe_w1 = np.asarray(inputs["oe_w1"], np.float32)
    pz_b1 = np.asarray(inputs["pz_b1"], np.float32)
    pr_b1 = np.asarray(inputs["pr_b1"], np.float32)
    oe_b1 = np.asarray(inputs["oe_b1"], np.float32)
    pz_b3 = np.asarray(inputs["pz_b3"], np.float32)
    oe_b3 = np.asarray(inputs["oe_b3"], np.float32)
    emb_a = np.asarray(inputs["pe_emb"], np.float32)[:Ka]
    b3m = pz_b3[:dL]

    # exp(g/TEMP) = x^-2  with x = -log(u+1e-10)+1e-10 (TEMP=0.5)
    x = (-np.log(u + np.float32(1e-10)) + np.float32(1e-10)).astype(np.float32)
    P = (1.0 / (x * x)).astype(np.float32)

    # t=0 regime softmax (active slice), rows layout
    e0 = np.exp(rl0 - rl0.max(axis=-1, keepdims=True))
    rp0 = (e0 / e0.sum(axis=-1, keepdims=True))[:, :, :Ka]  # (B,N,8)

    # per-(t,b) layer-1 bias pack + y' + qeps
    # l1 input rows: [h (dM), z (dL), e (dE)]
    qeps_const = (dL - 1) * 0.5 * LOG2PI

    def l1_bias(w1, b1v):
        bias_t = np.einsum('tbm,mh->tbh', h_seq, w1[:dM]) + b1v
        bias_t[1:] += w1[dM:dM + dL].T @ b3m   # carry excludes b3m for t>=1
        return bias_t.reshape(t_steps, B, 2, 128)  # (t, B, m, 128)

    bias_pz = l1_bias(pz_w1, pz_b1)
    bias_pr = l1_bias(pr_w1, pr_b1)
    # oe bias: h part rows oe_w1[dL:], + oe1_z^T b3m always
    bias_oe = np.einsum('tbm,mh->tbh', h_seq, oe_w1[dL:]) + oe_b1
    bias_oe += oe_w1[:dL].T @ b3m
    bias_oe = bias_oe.reshape(t_steps, B, 2, 128)

    yprime = obs - oe_b3[0]                     # (t, B)
    qeps = 0.5 * (eps.astype(np.float32) ** 2).sum(-1) + qeps_const  # (t,B,N)

    wmap = {
        "pz1_z": _f16(pz_w1[dM:dM + dL]),
        "pr1_z": _f16(pr_w1[dM:dM + dL]),
        "oe1_z": _f16(oe_w1[:dL]),
        "vpz": _f16(emb_a @ pz_w1[dM + dL:]),
        "vpr": _f16(emb_a @ pr_w1[dM + dL:]),
        "pz2": _f16(inputs["pz_w2"]), "pz3": _f16(inputs["pz_w3"]),
        "pr2": _f16(inputs["pr_w2"]),
        "oe2": _f16(inputs["oe_w2"]), "oe3": _f16(inputs["oe_w3"]),
        "b2z": np.asarray(inputs["pz_b2"], np.float32).reshape(2, 128).T.copy(),
        "b2o": np.asarray(inputs["oe_b2"], np.float32).reshape(1, 128).T.copy(),
        "b3s_rep": np.tile(_rep_row(pz_b3[dL:]), (1, NC)),
        "b3m_col": np.repeat(b3m[:, None], 1, axis=1).astype(np.float32),
        "oe3b1": np.full((128, 1), oe_b3[1], np.float32),
        "scales_rep": np.tile(_rep_row(np.log1p(np.exp(
            np.asarray(inputs["log_obs_scale"], np.float32)[:Ka]))), (1, NCB)),
        "prb2_rep": np.tile(_rep_row(np.asarray(inputs["pr_b2"], np.float32)),
                            (1, NCB)),
    }

    in_maps = []
    for core in range(NCORES):
        bp = [2 * core, 2 * core + 1]
        m = dict(wmap)
        m["pT"] = _bf16(P[:, bp].transpose(0, 1, 3, 2))       # (T,2,j,i)
        m["eps_n"] = _f16(eps[:, bp])                          # (T,2,N,dL)
        m["z0T"] = _f16(np.concatenate([z0[b].T for b in bp], axis=1))
        m["rp0T"] = _f16(np.concatenate([rp0[b].T for b in bp], axis=1))
        hbc = np.zeros((t_steps, 128, NHB), np.float32)
        for bi, bb in enumerate(bp):
            for mm in range(2):
                hbc[:, :, 0 * 4 + mm * 2 + bi] = bias_pz[:, bb, mm]
                hbc[:, :, 1 * 4 + mm * 2 + bi] = bias_pr[:, bb, mm]
                hbc[:, :, 2 * 4 + mm * 2 + bi] = bias_oe[:, bb, mm]
            hbc[:, :, 12 + bi] = yprime[:, bb, None]
            q = qeps[:, bb].reshape(t_steps, NC, 128)          # (t, jc, j)
            for jc in range(NC):
                hbc[:, :, 14 + bi * NC + jc] = q[:, jc]
        m["hb"] = hbc
        in_maps.append(m)
    return in_maps


def gather_output(results, t_steps=T_FULL):
    out = np.zeros((t_steps, B, dL), np.float32)
    for core in range(NCORES):
        r = results[core]["means"]                             # (2,128,T)
        for b in range(B2):
            out[:, 2 * core + b, :] = np.asarray(r[b], np.float32).T
    return out


def kernel(**inputs):
    from concourse.bass_utils import run_bass_kernel_spmd
    nc = build_core_program(T_FULL)
    in_maps = host_prep(inputs, T_FULL)
    res = run_bass_kernel_spmd(nc, in_maps, list(range(NCORES)))
    return gather_output(res.results, T_FULL)


if __name__ == "__main__":
    nc = build_core_program(2)
    print("built ok")


# revision 58
# speedup vs baseline: 1.2100x; 1.2100x over previous
"""Differentiable particle filter V3 — Trainium2 Bass kernel (optimized).

Data-parallel over batch B=16 across 8 NeuronCores (2 batch items/core).
Each core runs the T=16 sequential scan for its two particle clouds
(N=512, dL=128) with activations on-chip, feature-on-partition /
particle-on-free layout so MLP layers are PE matmul chains.

Optimizations vs v1:
- Host precomputes: per-(t,b) layer-1 biases (h contraction + b3m fold),
  y' = y - oe_b3[0], 0.5*sum(eps^2)+const, t=0 regime softmax, and
  V = emb @ W_e folding (kills remb matmuls and all on-device h work).
- 2 ACT table-set switches/step: exp(log_std) via tanh identity
  (tanh lives in the silu table set), whole tail uses natural_log_exp set.
- s_j folded into resample lhs (znew_s, rl9s); denominator rides g2 as a
  9th lhs column; 1/D broadcast via K=1 PE matmul (no DRAM bounce).
- Cross-partition max/sum via gpsimd.partition_all_reduce.
- 3 DMA descriptors per step issued from the idle sync engine.
- Resample skipped on the final step (carry unused).
"""

import numpy as np
import ml_dtypes

import concourse.bass as bass
import concourse.tile as tile
from concourse import mybir
from concourse import bass_isa
from concourse.masks import make_identity
from concourse.bass import ts

F32 = mybir.dt.float32
F32R = mybir.dt.float32r
F16 = mybir.dt.float16
BF16 = mybir.dt.bfloat16
AF = mybir.ActivationFunctionType
ALU = mybir.AluOpType
AX = mybir.AxisListType
RED = bass_isa.ReduceOp

LOG2PI = 1.8378770664093453

# problem dims (hardcoded per spec)
B, N, T_FULL = 16, 512, 16
dL, dM, dE, H = 128, 256, 32, 256
Kt, Ka = 18, 8
NCORES = 8
B2 = 2          # batch items per core
NC = 4          # 128-particle chunks per batch item
NCB = NC * B2   # particle chunks per core (8)
NT = B2 * N     # particles per core (1024)
NHB = 22        # host bias pack columns: 12 l1-bias, 2 y', 8 qeps


def split_waits(nc, limit=1):
    """This walrus build encodes at most one sync wait per instruction.
    Hoist excess waits onto injected same-engine NOPs placed immediately
    before the instruction (engine program order preserves semantics)."""
    for f in nc.m.functions:
        for bb in f.blocks:
            newl = []
            for ins in bb.instructions:
                si = ins.sync_info
                if si is not None and si.on_wait and len(si.on_wait) > limit:
                    waits = list(si.on_wait)
                    for k, wv in enumerate(waits[:-limit]):
                        nop = mybir.InstNoOp(
                            name=f"{ins.name}-ws{k}", ins=[], outs=[])
                        nop.engine = ins.engine
                        nop.sync_info = mybir.SyncInfo(on_wait=[wv], on_update=[])
                        newl.append(nop)
                    si.on_wait = waits[-limit:]
                newl.append(ins)
            try:
                bb.instructions = newl
            except Exception:
                bb.instructions.clear()
                bb.instructions.extend(newl)
    return nc


def build_core_program(t_steps=T_FULL):
    nc = bass.Bass()

    # ---------------- DRAM parameters (per-core shapes) ----------------
    d_pT = nc.declare_dram_parameter("pT", [t_steps, B2, N, N], BF16, isOutput=False)
    d_eps = nc.declare_dram_parameter("eps_n", [t_steps, B2, N, dL], F16, isOutput=False)
    d_hb = nc.declare_dram_parameter("hb", [t_steps, 128, NHB], F32, isOutput=False)
    d_z0T = nc.declare_dram_parameter("z0T", [dL, NT], F16, isOutput=False)
    d_rp0 = nc.declare_dram_parameter("rp0T", [Ka, NT], F16, isOutput=False)

    d_w = {}
    for name, shape, dt in [
        ("pz1_z", [dL, H], F16), ("pr1_z", [dL, H], F16), ("oe1_z", [dL, H], F16),
        ("vpz", [Ka, H], F16), ("vpr", [Ka, H], F16),
        ("pz2", [H, H], F16), ("pz3", [H, 2 * dL], F16),
        ("pr2", [H, Ka], F16),
        ("oe2", [H, 128], F16), ("oe3", [128, 2], F16),
        ("b2z", [128, 2], F32),          # pz_b2 cols
        ("b2o", [128, 1], F32),          # oe_b2 col
        ("b3s_rep", [128, NC * dL], F32),  # pz_b3[dL:] row replicated+tiled 4x
        ("b3m_col", [128, 1], F32),      # pz_b3[:dL] as partition col
        ("oe3b1", [128, 1], F32),        # oe_b3[1] replicated col
        ("scales_rep", [128, NCB * Ka], F32),  # softplus(scales) tiled 8x
        ("prb2_rep", [128, NCB * Ka], F32),    # pr_b2 tiled 8x
    ]:
        d_w[name] = nc.declare_dram_parameter(name, shape, dt, isOutput=False)

    d_out = nc.declare_dram_parameter("means", [B2, dL, t_steps], F32, isOutput=True)

    from contextlib import ExitStack
    with tile.TileContext(nc) as tc, ExitStack() as ctx:
        wp = ctx.enter_context(tc.tile_pool(name="wp", bufs=1))
        sp = ctx.enter_context(tc.tile_pool(name="sp", bufs=2))
        pp = ctx.enter_context(tc.tile_pool(name="pp", bufs=2))
        st = ctx.enter_context(tc.tile_pool(name="st", bufs=1))
        psA = ctx.enter_context(tc.tile_pool(name="psA", bufs=5, space="PSUM"))
        psC = ctx.enter_context(tc.tile_pool(name="psC", bufs=3, space="PSUM"))

        nV, nS, nG, nTe, nY = nc.vector, nc.scalar, nc.gpsimd, nc.tensor, nc.sync

        # ---------------- load weights / constants ----------------
        w = {}
        for wi, (name, h) in enumerate(d_w.items()):
            eng = nG if wi % 2 == 0 else nY
            shp = list(h.shape)
            if shp[0] > 128:
                assert shp[0] == 256
                tl = wp.tile([128, 2, shp[1]], h.dtype, name="w_" + name)
                eng.dma_start(out=tl, in_=h[:, :].rearrange("(c p) m -> p c m", p=128))
            else:
                tl = wp.tile(shp, h.dtype, name="w_" + name)
                eng.dma_start(out=tl, in_=h[:, :])
            w[name] = tl

        ident16 = wp.tile([128, 128], F16, name="ident16")
        make_identity(nc, ident16)
        ident32 = wp.tile([128, 128], F32, name="ident32")
        make_identity(nc, ident32)
        ones16 = wp.tile([1, 128], F16, name="ones16")
        nV.memset(ones16, 1.0)
        ones32 = wp.tile([1, 128], F32, name="ones32")
        nV.memset(ones32, 1.0)
        ones_bf = wp.tile([1, 128], BF16, name="ones_bf")
        nV.memset(ones_bf, 1.0)
        ones_r = wp.tile([1, 128], F32R, name="ones_r")
        nV.tensor_copy(out=ones_r, in_=ones32)


        # ---------------- state ----------------
        zT = st.tile([dL, NT], F16, name="zT_state")
        nG.dma_start(out=zT, in_=d_z0T[:, :])
        rp0 = st.tile([Ka, NT], F16, name="rp0_state")
        nG.dma_start(out=rp0, in_=d_rp0[:, :])
        # row 0 = D/D = 1 (ignored), rows 1..8 = normalized regime logits
        rlr_rows = st.tile([Ka + 1, NT], F32, name="rlr_rows")
        means_acc = st.tile([dL, B2, t_steps], F32, name="means_acc")

        # Collapse the weight/state-load DMA deps into one barrier (see
        # split_waits): a chain of tiny DVE reads accumulates every DMA
        # tick into DVE's vector clock; one NOP then covers all loads.
        from concourse.tile import add_dep_helper
        probe = st.tile([1, 1], F32, name="probe")
        last_copy = None
        for tl in [*w.values(), ident16, zT, rp0]:
            src = tl[0:1, 0, 0:1] if len(tl.shape) == 3 else tl[0:1, 0:1]
            last_copy = nV.tensor_copy(out=probe, in_=src)
        curr_bb = nc.cur_bb
        bar = nc.sync.nop()
        assert last_copy is not None
        add_dep_helper(bar.ins, last_copy.ins, sync=True, reason="weights barrier")
        tc.barrier_instruction_and_bb = (bar.ins, curr_bb)

        for t in range(t_steps):
            last = (t == t_steps - 1)
            # ---------------- step input DMAs (sync engine) ----------------
            if not last:
                P_t = pp.tile([128, NCB, N], BF16, name="P_t")
                nY.dma_start(out=P_t,
                             in_=d_pT[t].rearrange("b (c p) i -> p (b c) i", p=128))
            eps_t = sp.tile([128, NCB, dL], F16, name="eps_t")
            nY.dma_start(out=eps_t,
                         in_=d_eps[t].rearrange("b (c p) d -> p (b c) d", p=128))
            hb_t = sp.tile([128, NHB], F32, name="hb_t")
            nY.dma_start(out=hb_t, in_=d_hb[t])

            def l1bias(net, m, b):
                c = net * 4 + m * 2 + b
                return hb_t[:, c:c + 1]

            # pz layer-1 z-passes first: PE chews on these (psum left open,
            # start w/o stop) while the rl-softmax block runs on DVE/ACT.
            pz_zps = []
            for m in range(2):
                row = []
                for b in range(B2):
                    ps1 = psA.tile([128, N], F32, tag="ps", name="l1")
                    nTe.matmul(ps1, w["pz1_z"][:, ts(m, 128)], zT[:, ts(b, N)],
                               start=True, stop=False)
                    row.append(ps1)
                pz_zps.append(row)

            # ---------------- regime softmax -> rp_rows (8, NT) f16 -------
            # ACT exp here rides the ln_exp table set left over from the
            # previous step's tail (no switch).
            if t == 0:
                rp_rows = rp0
            else:
                rp_rows = sp.tile([Ka, NT], F16, name="rp_rows")
                rlc = sp.tile([128, NCB, Ka + 1], F32, name="rlc")
                for c in range(NCB):
                    tp = psC.tile([128, Ka + 1], F32, tag="ps", name="rltp")
                    nTe.transpose(tp, rlr_rows[:, ts(c, 128)],
                                  ident32[0:Ka + 1, 0:Ka + 1])
                    if c % 2 == 0:
                        nV.tensor_copy(out=rlc[:, c, :], in_=tp)
                    else:
                        nS.activation(out=rlc[:, c, :], in_=tp, func=AF.Copy)
                ernr = sp.tile([128, NCB, Ka], F32, name="ernr")
                nS.activation(out=ernr, in_=rlc[:, :, 1:Ka + 1], func=AF.Exp)
                dnr = sp.tile([128, NCB], F32, name="dnr")
                nV.tensor_reduce(out=dnr, in_=ernr, axis=AX.X, op=ALU.add)
                nV.tensor_scalar_add(dnr, dnr, float(Kt - Ka))
                rdnr = sp.tile([128, NCB], F32, name="rdnr")
                nV.reciprocal(out=rdnr, in_=dnr)
                rpc = sp.tile([128, NCB, Ka], F16, name="rpc")
                for c in range(NCB):
                    nV.tensor_scalar_mul(rpc[:, c, :], ernr[:, c, :],
                                         rdnr[:, c:c + 1])
                for c in range(NCB):
                    tpb = psC.tile([Ka, 128], F16, tag="ps", name="rptp")
                    nTe.transpose(tpb, rpc[:, c, :], ident16)
                    if c % 2 == 0:
                        nV.tensor_copy(out=rp_rows[:, ts(c, 128)], in_=tpb)
                    else:
                        nS.activation(out=rp_rows[:, ts(c, 128)], in_=tpb,
                                      func=AF.Copy)

            # ---------------- layer 1 (pz, pr) ----------------
            # z-passes for pz were already issued before the rl-softmax
            # block (PE overlap); close them with the e-pass then silu.
            def layer1(wz, ve, net, nm, zps=None):
                out = []
                for m in range(2):
                    ht = sp.tile([128, NT], F16, name=nm + str(m))
                    for b in range(B2):
                        if zps is None:
                            ps1 = psA.tile([128, N], F32, tag="ps", name="l1")
                            nTe.matmul(ps1, wz[:, ts(m, 128)], zT[:, ts(b, N)],
                                       start=True, stop=False)
                        else:
                            ps1 = zps[m][b]
                        nTe.matmul(ps1, ve[:, ts(m, 128)], rp_rows[:, ts(b, N)],
                                   start=False, stop=True)
                        nS.activation(out=ht[:, ts(b, N)], in_=ps1, func=AF.Silu,
                                      bias=l1bias(net, m, b))
                    out.append(ht)
                return out

            hz1 = layer1(w["pz1_z"], w["vpz"], 0, "hz1_", zps=pz_zps)
            prh = layer1(w["pr1_z"], w["vpr"], 1, "prh_")

            # ---------------- layer 2 (pz2) ----------------
            hz2 = []
            for m in range(2):
                ht = sp.tile([128, NT], F16, name="hz2_" + str(m))
                for b in range(B2):
                    ps2 = psA.tile([128, N], F32, tag="ps", name="l2")
                    nTe.matmul(ps2, w["pz2"][:, 0, ts(m, 128)], hz1[0][:, ts(b, N)],
                               start=True, stop=False)
                    nTe.matmul(ps2, w["pz2"][:, 1, ts(m, 128)], hz1[1][:, ts(b, N)],
                               start=False, stop=True)
                    nS.activation(out=ht[:, ts(b, N)], in_=ps2, func=AF.Silu,
                                  bias=w["b2z"][:, m:m + 1])
                hz2.append(ht)

            # ---------------- pz3 (flip) + znew + log-q pieces -------------
            znew16 = sp.tile([128, NCB, dL], F16, name="znew16")
            qls = sp.tile([128, NCB], F32, name="qls")
            NH = 2  # chunks per pz3 psum tile
            for b in range(B2):
                for h2 in range(NC // NH):
                    c0 = b * NC + h2 * NH
                    zps = psA.tile([128, NH, 2 * dL], F32, tag="ps", name="zp")
                    for jj in range(NH):
                        nTe.matmul(zps[:, jj, :], hz2[0][:, ts(c0 + jj, 128)],
                                   w["pz3"][:, 0, :], start=True, stop=False)
                        nTe.matmul(zps[:, jj, :], hz2[1][:, ts(c0 + jj, 128)],
                                   w["pz3"][:, 1, :], start=False, stop=True)
                    # ls = clip(raw + b3s, -5, 2); qls = sum_d ls
                    lsa = sp.tile([128, NH, dL], F32, name="lsa")
                    nV.tensor_tensor(out=lsa, in0=zps[:, :, dL:2 * dL],
                                     in1=w["b3s_rep"][:, 0:NH * dL].rearrange(
                                         "p (c d) -> p c d", c=NH),
                                     op=ALU.add)
                    nV.tensor_scalar(lsa, lsa, 2.0, -5.0,
                                     op0=ALU.min, op1=ALU.max)
                    nV.tensor_reduce(out=qls[:, c0:c0 + NH], in_=lsa,
                                     axis=AX.X, op=ALU.add)
                    els = sp.tile([128, NH, dL], F32, name="els")
                    nS.activation(out=els, in_=lsa, func=AF.Exp)
                    p1 = sp.tile([128, NH, dL], F32, name="p1")
                    nV.tensor_mul(p1, els, eps_t[:, c0:c0 + NH, :])
                    nV.tensor_tensor(out=znew16[:, c0:c0 + NH, :], in0=p1,
                                     in1=zps[:, :, 0:dL], op=ALU.add)

            # ---------------- znT transpose ----------------
            znT = sp.tile([dL, NT], F16, name="znT")
            for c in range(NCB):
                tps = psC.tile([128, 128], F16, tag="ps", name="ztp")
                nTe.transpose(tps, znew16[:, c, :], ident16)
                if c % 2 == 0:
                    nV.tensor_copy(out=znT[:, ts(c, 128)], in_=tps)
                else:
                    nS.activation(out=znT[:, ts(c, 128)], in_=tps, func=AF.Copy)

            # ---------------- observation net ----------------
            oeh = []
            for m in range(2):
                ht = sp.tile([128, NT], F16, name="oeh_" + str(m))
                for b in range(B2):
                    pso = psA.tile([128, N], F32, tag="ps", name="o1")
                    nTe.matmul(pso, w["oe1_z"][:, ts(m, 128)], znT[:, ts(b, N)],
                               start=True, stop=True)
                    nS.activation(out=ht[:, ts(b, N)], in_=pso, func=AF.Silu,
                                  bias=l1bias(2, m, b))
                oeh.append(ht)
            em2 = sp.tile([128, NT], F16, name="em2")
            for b in range(B2):
                pso = psA.tile([128, N], F32, tag="ps", name="o2")
                nTe.matmul(pso, w["oe2"][:, 0, :], oeh[0][:, ts(b, N)],
                           start=True, stop=False)
                nTe.matmul(pso, w["oe2"][:, 1, :], oeh[1][:, ts(b, N)],
                           start=False, stop=True)
                nS.activation(out=em2[:, ts(b, N)], in_=pso, func=AF.Silu,
                              bias=w["b2o"][:, 0:1])
            o3ps = psC.tile([128, NCB, 2], F32, tag="ps", name="o3")
            for c in range(NCB):
                nTe.matmul(o3ps[:, c, :], em2[:, ts(c, 128)], w["oe3"],
                           start=True, stop=True)

            # ---------------- pr layer-2 (flip) ----------------
            r2ps = psC.tile([128, NCB, Ka], F32, tag="ps", name="pr2")
            for c in range(NCB):
                nTe.matmul(r2ps[:, c, :], prh[0][:, ts(c, 128)], w["pr2"][:, 0, :],
                           start=True, stop=False)
                nTe.matmul(r2ps[:, c, :], prh[1][:, ts(c, 128)], w["pr2"][:, 1, :],
                           start=False, stop=True)
            rlog = sp.tile([128, NCB, Ka], F32, name="rlog")
            nV.tensor_tensor(out=rlog, in0=r2ps,
                             in1=w["prb2_rep"][:, :].rearrange(
                                 "p (c k) -> p c k", c=NCB), op=ALU.add)

            # ======= tail (natural_log_exp table set from here) =======
            ern = sp.tile([128, NCB, Ka], F32, name="ern")
            nS.activation(out=ern, in_=rlog, func=AF.Exp)
            u_t = sp.tile([128, NCB], F32, name="u_t")
            nS.activation(out=u_t, in_=o3ps[:, :, 1], func=AF.Exp,
                          bias=w["oe3b1"][:, 0:1])
            spv = sp.tile([128, NCB], F32, name="spv")
            nS.activation(out=spv, in_=u_t, func=AF.Ln, bias=1.0)  # softplus

            dn = sp.tile([128, NCB], F32, name="dn")
            nV.tensor_reduce(out=dn, in_=ern, axis=AX.X, op=ALU.add)
            nV.tensor_scalar_add(dn, dn, float(Kt - Ka))
            rdn = sp.tile([128, NCB], F32, name="rdn")
            nV.reciprocal(out=rdn, in_=dn)
            smu = sp.tile([128, NCB, Ka], F32, name="smu")
            nV.tensor_tensor(out=smu, in0=ern,
                             in1=w["scales_rep"][:, :].rearrange(
                                 "p (c k) -> p c k", c=NCB), op=ALU.mult)
            smult = sp.tile([128, NCB], F32, name="smult")
            nV.tensor_reduce(out=smult, in_=smu, axis=AX.X, op=ALU.add)
            nV.tensor_mul(smult, smult, rdn)

            sig = sp.tile([128, NCB], F32, name="sig")
            nV.tensor_mul(sig, spv, smult)
            nV.tensor_scalar(sig, sig, 5.0, 0.1, op0=ALU.min, op1=ALU.max)
            rsig = sp.tile([128, NCB], F32, name="rsig")
            nV.reciprocal(out=rsig, in_=sig)
            dev = sp.tile([128, NCB], F32, name="dev")
            for b in range(B2):
                nV.tensor_scalar_sub(dev[:, b * NC:(b + 1) * NC],
                                     o3ps[:, b * NC:(b + 1) * NC, 0],
                                     hb_t[:, 12 + b:13 + b])
            nV.tensor_mul(dev, dev, rsig)
            sq = sp.tile([128, NCB], F32, name="sq")
            nV.tensor_mul(sq, dev, dev)
            qtot = sp.tile([128, NCB], F32, name="qtot")
            nV.tensor_tensor(out=qtot, in0=qls, in1=hb_t[:, 14:22], op=ALU.add)
            # lw = lwn - ln(sig): keep the ln implicit (exp(-ln sig) = rsig).
            # M' = max(lwn) + ln(5) >= max(lw) since -ln(sig) <= ln(1/0.1).
            lwn = sp.tile([128, NCB], F32, name="lwn")
            nV.scalar_tensor_tensor(out=lwn, in0=sq, scalar=-0.5, in1=qtot,
                                    op0=ALU.mult, op1=ALU.add)

            mxc = sp.tile([128, B2], F32, name="mxc")
            for b in range(B2):
                nV.tensor_reduce(out=mxc[:, b:b + 1],
                                 in_=lwn[:, b * NC:(b + 1) * NC],
                                 axis=AX.X, op=ALU.max)
            mrow = sp.tile([1, B2], F32, name="mrow")
            nG.tensor_reduce(out=mrow, in_=mxc, axis=AX.C, op=ALU.max)
            nG.tensor_scalar(mrow, mrow, -1.0, -2.302586, op0=ALU.mult,
                             op1=ALU.add)
            nmb_ps = psC.tile([128, B2], F32, tag="ps", name="nmb_ps")
            nTe.matmul(nmb_ps, ones32, mrow, start=True, stop=True)
            nmb = sp.tile([128, B2], F32, name="nmb")
            nV.tensor_copy(out=nmb, in_=nmb_ps)
            lwm = sp.tile([128, NCB], F32, name="lwm")
            for b in range(B2):
                nV.tensor_scalar_add(lwm[:, b * NC:(b + 1) * NC],
                                     lwn[:, b * NC:(b + 1) * NC],
                                     nmb[:, b:b + 1])
            ewp = sp.tile([128, NCB], F32, name="ewp")
            nS.activation(out=ewp, in_=lwm, func=AF.Exp)
            e_w = sp.tile([128, NCB], F32, name="e_w")
            nV.tensor_mul(e_w, ewp, rsig)
            ew16 = sp.tile([128, NCB], F16, name="ew16")
            nV.tensor_copy(out=ew16, in_=e_w)

            # weighted-mean output
            sw = sp.tile([128, B2], F32, name="sw")
            for b in range(B2):
                nV.tensor_reduce(out=sw[:, b:b + 1],
                                 in_=e_w[:, b * NC:(b + 1) * NC],
                                 axis=AX.X, op=ALU.add)
            swrow = sp.tile([1, B2], F32, name="swrow")
            nG.tensor_reduce(out=swrow, in_=sw, axis=AX.C, op=ALU.add)
            swa_ps = psC.tile([128, B2], F32, tag="ps", name="swa_ps")
            nTe.matmul(swa_ps, ones32, swrow, start=True, stop=True)
            rse = sp.tile([128, B2], F32, name="rse")
            nV.reciprocal(out=rse, in_=swa_ps)
            for b in range(B2):
                mz = psC.tile([128, 1], F32, tag="ps", name="mz")
                for jc in range(NC):
                    c = b * NC + jc
                    nTe.matmul(mz, znew16[:, c, :], ew16[:, c:c + 1],
                               start=(jc == 0), stop=(jc == NC - 1))
                nV.scalar_tensor_tensor(out=means_acc[:, b, t:t + 1], in0=mz,
                                        scalar=rse[:, b:b + 1],
                                        in1=w["b3m_col"],
                                        op0=ALU.mult, op1=ALU.add)

            # ---------------- soft resample (skipped on last step) --------
            if last:
                continue
            sj = sp.tile([128, NCB], F32, name="sj")
            nV.tensor_mul(sj, e_w, e_w)  # exp(2*(lw-M))
            zs = sp.tile([128, NCB, dL], BF16, name="zs")
            rl9s = sp.tile([128, NCB, Ka + 1], BF16, name="rl9s")
            nV.tensor_copy(out=rl9s[:, :, 0], in_=sj)
            g1s, g2s = [], []
            for b in range(B2):
                # per-b prep on gpsimd (sbuf-only) so PE can start b0's
                # matmuls while b1's prep still runs
                for jc in range(NC):
                    c = b * NC + jc
                    nV.tensor_scalar_mul(zs[:, c, :], znew16[:, c, :],
                                         sj[:, c:c + 1])
                    nV.tensor_scalar_mul(rl9s[:, c, 1:Ka + 1], rlog[:, c, :],
                                         sj[:, c:c + 1])
                g1 = psA.tile([dL, N], F32, tag="ps", name="g1")
                g2 = psA.tile([Ka + 1, N], F32, tag="ps", name="g2")
                for jc in range(NC):
                    c = b * NC + jc
                    nTe.matmul(g1, zs[:, c, :], P_t[:, c, :],
                               start=(jc == 0), stop=(jc == NC - 1))
                    nTe.matmul(g2, rl9s[:, c, :], P_t[:, c, :],
                               start=(jc == 0), stop=(jc == NC - 1))
                g1s.append(g1)
                g2s.append(g2)
                # 1/D row via Ln -> Exp(-1) on ACT (ln_exp set, no switch);
                # f32r PE broadcast (1 cyc/row), one psum->sbuf copy.
                dln = sp.tile([1, N], F32, name="dln")
                nS.activation(out=dln, in_=g2[0:1, :], func=AF.Ln)
                rdr = sp.tile([1, N], F32R, name="rdr")
                nS.activation(out=rdr, in_=dln, func=AF.Exp, scale=-1.0)
                rdbc = psA.tile([128, N], F32, tag="ps", name="rdbc")
                nTe.matmul(rdbc, ones_r, rdr, start=True, stop=True)
                rdsb = sp.tile([128, N], F32, name="rdsb")
                nV.tensor_copy(out=rdsb, in_=rdbc)
                nV.tensor_mul(zT[:, ts(b, N)], g1, rdsb)
                nV.tensor_mul(rlr_rows[:, ts(b, N)], g2,
                              rdsb[0:Ka + 1, :])

        # ---------------- write outputs ----------------
        for b in range(B2):
            nY.dma_start(out=d_out[b], in_=means_acc[:, b, :])

    return split_waits(nc)


# ======================= host side =======================

def _f16(x):
    return np.asarray(x, np.float32).astype(np.float16)


def _bf16(x):
    return np.asarray(x, np.float32).astype(ml_dtypes.bfloat16)


def _rep_row(row, p=128):
    """replicate a row vector across 128 partitions."""
    r = np.asarray(row, np.float32).reshape(-1)
    return np.broadcast_to(r[None, :], (p, r.shape[0])).copy()


def host_prep(inputs, t_steps=T_FULL):
    obs = np.asarray(inputs["obs"], np.float32)[:t_steps]
    h_seq = np.asarray(inputs["h_seq"], np.float32)[:t_steps]
    z0 = np.asarray(inputs["z0"], np.float32)
    rl0 = np.asarray(inputs["regime_logits0"], np.float32)
    eps = np.asarray(inputs["eps"], np.float32)[:t_steps]
    u = np.asarray(inputs["gumbel_u"], np.float32)[:t_steps]
    assert int(inputs["k_active"]) == Ka

    pz_w1 = np.asarray(inputs["pz_w1"], np.float32)
    pr_w1 = np.asarray(inputs["pr_w1"], np.float32)
    oe_w1 = np.asarray(inputs["oe_w1"], np.float32)
    pz_b1 = np.asarray(inputs["pz_b1"], np.float32)
    pr_b1 = np.asarray(inputs["pr_b1"], np.float32)
    oe_b1 = np.asarray(inputs["oe_b1"], np.float32)
    pz_b3 = np.asarray(inputs["pz_b3"], np.float32)
    oe_b3 = np.asarray(inputs["oe_b3"], np.float32)
    emb_a = np.asarray(inputs["pe_emb"], np.float32)[:Ka]
    b3m = pz_b3[:dL]

    # exp(g/TEMP) = x^-2  with x = -log(u+1e-10)+1e-10 (TEMP=0.5)
    x = (-np.log(u + np.float32(1e-10)) + np.float32(1e-10)).astype(np.float32)
    P = (1.0 / (x * x)).astype(np.float32)

    # t=0 regime softmax (active slice), rows layout
    e0 = np.exp(rl0 - rl0.max(axis=-1, keepdims=True))
    rp0 = (e0 / e0.sum(axis=-1, keepdims=True))[:, :, :Ka]  # (B,N,8)

    # per-(t,b) layer-1 bias pack + y' + qeps
    # l1 input rows: [h (dM), z (dL), e (dE)]
    qeps_const = (dL - 1) * 0.5 * LOG2PI

    def l1_bias(w1, b1v):
        bias_t = np.einsum('tbm,mh->tbh', h_seq, w1[:dM]) + b1v
        bias_t[1:] += w1[dM:dM + dL].T @ b3m   # carry excludes b3m for t>=1
        return bias_t.reshape(t_steps, B, 2, 128)  # (t, B, m, 128)

    bias_pz = l1_bias(pz_w1, pz_b1)
    bias_pr = l1_bias(pr_w1, pr_b1)
    # oe bias: h part rows oe_w1[dL:], + oe1_z^T b3m always
    bias_oe = np.einsum('tbm,mh->tbh', h_seq, oe_w1[dL:]) + oe_b1
    bias_oe += oe_w1[:dL].T @ b3m
    bias_oe = bias_oe.reshape(t_steps, B, 2, 128)

    yprime = obs - oe_b3[0]                     # (t, B)
    qeps = 0.5 * (eps.astype(np.float32) ** 2).sum(-1) + qeps_const  # (t,B,N)

    wmap = {
        "pz1_z": _f16(pz_w1[dM:dM + dL]),
        "pr1_z": _f16(pr_w1[dM:dM + dL]),
        "oe1_z": _f16(oe_w1[:dL]),
        "vpz": _f16(emb_a @ pz_w1[dM + dL:]),
        "vpr": _f16(emb_a @ pr_w1[dM + dL:]),
        "pz2": _f16(inputs["pz_w2"]), "pz3": _f16(inputs["pz_w3"]),
        "pr2": _f16(inputs["pr_w2"]),
        "oe2": _f16(inputs["oe_w2"]), "oe3": _f16(inputs["oe_w3"]),
        "b2z": np.asarray(inputs["pz_b2"], np.float32).reshape(2, 128).T.copy(),
        "b2o": np.asarray(inputs["oe_b2"], np.float32).reshape(1, 128).T.copy(),
        "b3s_rep": np.tile(_rep_row(pz_b3[dL:]), (1, NC)),
        "b3m_col": np.repeat(b3m[:, None], 1, axis=1).astype(np.float32),
        "oe3b1": np.full((128, 1), oe_b3[1], np.float32),
        "scales_rep": np.tile(_rep_row(np.log1p(np.exp(
            np.asarray(inputs["log_obs_scale"], np.float32)[:Ka]))), (1, NCB)),
        "prb2_rep": np.tile(_rep_row(np.asarray(inputs["pr_b2"], np.float32)),
                            (1, NCB)),
    }

    in_maps = []
    for core in range(NCORES):
        bp = [2 * core, 2 * core + 1]
        m = dict(wmap)
        m["pT"] = _bf16(P[:, bp].transpose(0, 1, 3, 2))       # (T,2,j,i)
        m["eps_n"] = _f16(eps[:, bp])                          # (T,2,N,dL)
        m["z0T"] = _f16(np.concatenate([z0[b].T for b in bp], axis=1))
        m["rp0T"] = _f16(np.concatenate([rp0[b].T for b in bp], axis=1))
        hbc = np.zeros((t_steps, 128, NHB), np.float32)
        for bi, bb in enumerate(bp):
            for mm in range(2):
                hbc[:, :, 0 * 4 + mm * 2 + bi] = bias_pz[:, bb, mm]
                hbc[:, :, 1 * 4 + mm * 2 + bi] = bias_pr[:, bb, mm]
                hbc[:, :, 2 * 4 + mm * 2 + bi] = bias_oe[:, bb, mm]
            hbc[:, :, 12 + bi] = yprime[:, bb, None]
            q = qeps[:, bb].reshape(t_steps, NC, 128)          # (t, jc, j)
            for jc in range(NC):
                hbc[:, :, 14 + bi * NC + jc] = q[:, jc]
        m["hb"] = hbc
        in_maps.append(m)
    return in_maps


def gather_output(results, t_steps=T_FULL):
    out = np.zeros((t_steps, B, dL), np.float32)
    for core in range(NCORES):
        r = results[core]["means"]                             # (2,128,T)
        for b in range(B2):
            out[:, 2 * core + b, :] = np.asarray(r[b], np.float32).T
    return out


def kernel(**inputs):
    from concourse.bass_utils import run_bass_kernel_spmd
    nc = build_core_program(T_FULL)
    in_maps = host_prep(inputs, T_FULL)
    res = run_bass_kernel_spmd(nc, in_maps, list(range(NCORES)))
    return gather_output(res.results, T_FULL)


if __name__ == "__main__":
    nc = build_core_program(2)
    print("built ok")


# revision 59
# speedup vs baseline: 1.2278x; 1.0147x over previous
"""Differentiable particle filter V3 — Trainium2 Bass kernel (optimized).

Data-parallel over batch B=16 across 8 NeuronCores (2 batch items/core).
Each core runs the T=16 sequential scan for its two particle clouds
(N=512, dL=128) with activations on-chip, feature-on-partition /
particle-on-free layout so MLP layers are PE matmul chains.

Optimizations vs v1:
- Host precomputes: per-(t,b) layer-1 biases (h contraction + b3m fold),
  y' = y - oe_b3[0], 0.5*sum(eps^2)+const, t=0 regime softmax, and
  V = emb @ W_e folding (kills remb matmuls and all on-device h work).
- 2 ACT table-set switches/step: exp(log_std) via tanh identity
  (tanh lives in the silu table set), whole tail uses natural_log_exp set.
- s_j folded into resample lhs (znew_s, rl9s); denominator rides g2 as a
  9th lhs column; 1/D broadcast via K=1 PE matmul (no DRAM bounce).
- Cross-partition max/sum via gpsimd.partition_all_reduce.
- 3 DMA descriptors per step issued from the idle sync engine.
- Resample skipped on the final step (carry unused).
"""

import numpy as np
import ml_dtypes

import concourse.bass as bass
import concourse.tile as tile
from concourse import mybir
from concourse import bass_isa
from concourse.masks import make_identity
from concourse.bass import ts

F32 = mybir.dt.float32
F32R = mybir.dt.float32r
F16 = mybir.dt.float16
BF16 = mybir.dt.bfloat16
AF = mybir.ActivationFunctionType
ALU = mybir.AluOpType
AX = mybir.AxisListType
RED = bass_isa.ReduceOp

LOG2PI = 1.8378770664093453

# problem dims (hardcoded per spec)
B, N, T_FULL = 16, 512, 16
dL, dM, dE, H = 128, 256, 32, 256
Kt, Ka = 18, 8
NCORES = 8
B2 = 2          # batch items per core
NC = 4          # 128-particle chunks per batch item
NCB = NC * B2   # particle chunks per core (8)
NT = B2 * N     # particles per core (1024)
NHB = 22        # host bias pack columns: 12 l1-bias, 2 y', 8 qeps


def split_waits(nc, limit=1):
    """This walrus build encodes at most one sync wait per instruction.
    Hoist excess waits onto injected same-engine NOPs placed immediately
    before the instruction (engine program order preserves semantics)."""
    for f in nc.m.functions:
        for bb in f.blocks:
            newl = []
            for ins in bb.instructions:
                si = ins.sync_info
                if si is not None and si.on_wait and len(si.on_wait) > limit:
                    waits = list(si.on_wait)
                    for k, wv in enumerate(waits[:-limit]):
                        nop = mybir.InstNoOp(
                            name=f"{ins.name}-ws{k}", ins=[], outs=[])
                        nop.engine = ins.engine
                        nop.sync_info = mybir.SyncInfo(on_wait=[wv], on_update=[])
                        newl.append(nop)
                    si.on_wait = waits[-limit:]
                newl.append(ins)
            try:
                bb.instructions = newl
            except Exception:
                bb.instructions.clear()
                bb.instructions.extend(newl)
    return nc


def build_core_program(t_steps=T_FULL):
    nc = bass.Bass()

    # ---------------- DRAM parameters (per-core shapes) ----------------
    d_pT = nc.declare_dram_parameter("pT", [t_steps, B2, N, N], BF16, isOutput=False)
    d_eps = nc.declare_dram_parameter("eps_n", [t_steps, B2, N, dL], F16, isOutput=False)
    d_hb = nc.declare_dram_parameter("hb", [t_steps, 128, NHB], F32, isOutput=False)
    d_z0T = nc.declare_dram_parameter("z0T", [dL, NT], F16, isOutput=False)
    d_rp0 = nc.declare_dram_parameter("rp0T", [Ka, NT], F16, isOutput=False)

    d_w = {}
    for name, shape, dt in [
        ("pz1_z", [dL, H], F16), ("pr1_z", [dL, H], F16), ("oe1_z", [dL, H], F16),
        ("vpz", [Ka, H], F16), ("vpr", [Ka, H], F16),
        ("pz2", [H, H], F16), ("pz3", [H, 2 * dL], F16),
        ("pr2", [H, Ka], F16),
        ("oe2", [H, 128], F16), ("oe3", [128, 2], F16),
        ("b2z", [128, 2], F32),          # pz_b2 cols
        ("b2o", [128, 1], F32),          # oe_b2 col
        ("b3s_rep", [128, NC * dL], F32),  # pz_b3[dL:] row replicated+tiled 4x
        ("b3m_col", [128, 1], F32),      # pz_b3[:dL] as partition col
        ("oe3b1", [128, 1], F32),        # oe_b3[1] replicated col
        ("scales_rep", [128, NCB * Ka], F32),  # softplus(scales) tiled 8x
        ("prb2_rep", [128, NCB * Ka], F32),    # pr_b2 tiled 8x
    ]:
        d_w[name] = nc.declare_dram_parameter(name, shape, dt, isOutput=False)

    d_out = nc.declare_dram_parameter("means", [B2, dL, t_steps], F32, isOutput=True)

    from contextlib import ExitStack
    with tile.TileContext(nc) as tc, ExitStack() as ctx:
        wp = ctx.enter_context(tc.tile_pool(name="wp", bufs=1))
        sp = ctx.enter_context(tc.tile_pool(name="sp", bufs=2))
        pp = ctx.enter_context(tc.tile_pool(name="pp", bufs=2))
        st = ctx.enter_context(tc.tile_pool(name="st", bufs=1))
        psA = ctx.enter_context(tc.tile_pool(name="psA", bufs=4, space="PSUM"))
        psC = ctx.enter_context(tc.tile_pool(name="psC", bufs=4, space="PSUM"))

        nV, nS, nG, nTe, nY = nc.vector, nc.scalar, nc.gpsimd, nc.tensor, nc.sync

        # ---------------- load weights / constants ----------------
        w = {}
        for wi, (name, h) in enumerate(d_w.items()):
            eng = nG if wi % 2 == 0 else nY
            shp = list(h.shape)
            if shp[0] > 128:
                assert shp[0] == 256
                tl = wp.tile([128, 2, shp[1]], h.dtype, name="w_" + name)
                eng.dma_start(out=tl, in_=h[:, :].rearrange("(c p) m -> p c m", p=128))
            else:
                tl = wp.tile(shp, h.dtype, name="w_" + name)
                eng.dma_start(out=tl, in_=h[:, :])
            w[name] = tl

        ident16 = wp.tile([128, 128], F16, name="ident16")
        make_identity(nc, ident16)
        ident32 = wp.tile([128, 128], F32, name="ident32")
        make_identity(nc, ident32)
        ones16 = wp.tile([1, 128], F16, name="ones16")
        nV.memset(ones16, 1.0)
        ones32 = wp.tile([1, 128], F32, name="ones32")
        nV.memset(ones32, 1.0)
        ones_bf = wp.tile([1, 128], BF16, name="ones_bf")
        nV.memset(ones_bf, 1.0)
        ones_r = wp.tile([1, 128], F32R, name="ones_r")
        nV.tensor_copy(out=ones_r, in_=ones32)


        # ---------------- state ----------------
        zT = st.tile([dL, NT], F16, name="zT_state")
        nG.dma_start(out=zT, in_=d_z0T[:, :])
        rp0 = st.tile([Ka, NT], F16, name="rp0_state")
        nG.dma_start(out=rp0, in_=d_rp0[:, :])
        # row 0 = D/D = 1 (ignored), rows 1..8 = normalized regime logits
        rlr_rows = st.tile([Ka + 1, NT], F32, name="rlr_rows")
        means_acc = st.tile([dL, B2, t_steps], F32, name="means_acc")

        # Collapse the weight/state-load DMA deps into one barrier (see
        # split_waits): a chain of tiny DVE reads accumulates every DMA
        # tick into DVE's vector clock; one NOP then covers all loads.
        from concourse.tile import add_dep_helper
        probe = st.tile([1, 1], F32, name="probe")
        last_copy = None
        for tl in [*w.values(), ident16, zT, rp0]:
            src = tl[0:1, 0, 0:1] if len(tl.shape) == 3 else tl[0:1, 0:1]
            last_copy = nV.tensor_copy(out=probe, in_=src)
        curr_bb = nc.cur_bb
        bar = nc.sync.nop()
        assert last_copy is not None
        add_dep_helper(bar.ins, last_copy.ins, sync=True, reason="weights barrier")
        tc.barrier_instruction_and_bb = (bar.ins, curr_bb)

        for t in range(t_steps):
            last = (t == t_steps - 1)
            # ---------------- step input DMAs (sync engine) ----------------
            if not last:
                P_t = pp.tile([128, NCB, N], BF16, name="P_t")
                nY.dma_start(out=P_t,
                             in_=d_pT[t].rearrange("b (c p) i -> p (b c) i", p=128))
            eps_t = sp.tile([128, NCB, dL], F16, name="eps_t")
            nY.dma_start(out=eps_t,
                         in_=d_eps[t].rearrange("b (c p) d -> p (b c) d", p=128))
            hb_t = sp.tile([128, NHB], F32, name="hb_t")
            nY.dma_start(out=hb_t, in_=d_hb[t])

            def l1bias(net, m, b):
                c = net * 4 + m * 2 + b
                return hb_t[:, c:c + 1]

            # pz layer-1 z-passes first: PE chews on these (psum left open,
            # start w/o stop) while the rl-softmax block runs on DVE/ACT.
            pz_zps = []
            for m in range(2):
                row = []
                for b in range(B2):
                    ps1 = psA.tile([128, N], F32, tag="ps", name="l1")
                    nTe.matmul(ps1, w["pz1_z"][:, ts(m, 128)], zT[:, ts(b, N)],
                               start=True, stop=False)
                    row.append(ps1)
                pz_zps.append(row)

            # ---------------- regime softmax -> rp_rows (8, NT) f16 -------
            # ACT exp here rides the ln_exp table set left over from the
            # previous step's tail (no switch).
            if t == 0:
                rp_rows = rp0
            else:
                rp_rows = sp.tile([Ka, NT], F16, name="rp_rows")
                rlc = sp.tile([128, NCB, Ka + 1], F32, name="rlc")
                for c in range(NCB):
                    tp = psC.tile([128, Ka + 1], F32, tag="ps", name="rltp")
                    nTe.transpose(tp, rlr_rows[:, ts(c, 128)],
                                  ident32[0:Ka + 1, 0:Ka + 1])
                    if c % 2 == 0:
                        nV.tensor_copy(out=rlc[:, c, :], in_=tp)
                    else:
                        nS.activation(out=rlc[:, c, :], in_=tp, func=AF.Copy)
                ernr = sp.tile([128, NCB, Ka], F32, name="ernr")
                nS.activation(out=ernr, in_=rlc[:, :, 1:Ka + 1], func=AF.Exp)
                dnr = sp.tile([128, NCB], F32, name="dnr")
                nV.tensor_reduce(out=dnr, in_=ernr, axis=AX.X, op=ALU.add)
                nV.tensor_scalar_add(dnr, dnr, float(Kt - Ka))
                rdnr = sp.tile([128, NCB], F32, name="rdnr")
                nV.reciprocal(out=rdnr, in_=dnr)
                rpc = sp.tile([128, NCB, Ka], F16, name="rpc")
                for c in range(NCB):
                    nV.tensor_scalar_mul(rpc[:, c, :], ernr[:, c, :],
                                         rdnr[:, c:c + 1])
                for c in range(NCB):
                    tpb = psC.tile([Ka, 128], F16, tag="ps", name="rptp")
                    nTe.transpose(tpb, rpc[:, c, :], ident16)
                    if c % 2 == 0:
                        nV.tensor_copy(out=rp_rows[:, ts(c, 128)], in_=tpb)
                    else:
                        nS.activation(out=rp_rows[:, ts(c, 128)], in_=tpb,
                                      func=AF.Copy)

            # ---------------- layer 1 (pz, pr) ----------------
            # z-passes for pz were already issued before the rl-softmax
            # block (PE overlap); close them with the e-pass then silu.
            def layer1(wz, ve, net, nm, zps=None):
                out = []
                for m in range(2):
                    ht = sp.tile([128, NT], F16, name=nm + str(m))
                    for b in range(B2):
                        if zps is None:
                            ps1 = psA.tile([128, N], F32, tag="ps", name="l1")
                            nTe.matmul(ps1, wz[:, ts(m, 128)], zT[:, ts(b, N)],
                                       start=True, stop=False)
                        else:
                            ps1 = zps[m][b]
                        nTe.matmul(ps1, ve[:, ts(m, 128)], rp_rows[:, ts(b, N)],
                                   start=False, stop=True)
                        nS.activation(out=ht[:, ts(b, N)], in_=ps1, func=AF.Silu,
                                      bias=l1bias(net, m, b))
                    out.append(ht)
                return out

            hz1 = layer1(w["pz1_z"], w["vpz"], 0, "hz1_", zps=pz_zps)
            prh = layer1(w["pr1_z"], w["vpr"], 1, "prh_")

            # ---------------- layer 2 (pz2) ----------------
            hz2 = []
            for m in range(2):
                ht = sp.tile([128, NT], F16, name="hz2_" + str(m))
                for b in range(B2):
                    ps2 = psA.tile([128, N], F32, tag="ps", name="l2")
                    nTe.matmul(ps2, w["pz2"][:, 0, ts(m, 128)], hz1[0][:, ts(b, N)],
                               start=True, stop=False)
                    nTe.matmul(ps2, w["pz2"][:, 1, ts(m, 128)], hz1[1][:, ts(b, N)],
                               start=False, stop=True)
                    nS.activation(out=ht[:, ts(b, N)], in_=ps2, func=AF.Silu,
                                  bias=w["b2z"][:, m:m + 1])
                hz2.append(ht)

            # ---------------- pz3 (flip) + znew + log-q pieces -------------
            znew16 = sp.tile([128, NCB, dL], F16, name="znew16")
            qls = sp.tile([128, NCB], F32, name="qls")
            NH = 2  # chunks per pz3 psum tile
            for b in range(B2):
                for h2 in range(NC // NH):
                    c0 = b * NC + h2 * NH
                    zps = psA.tile([128, NH, 2 * dL], F32, tag="ps", name="zp")
                    for jj in range(NH):
                        nTe.matmul(zps[:, jj, :], hz2[0][:, ts(c0 + jj, 128)],
                                   w["pz3"][:, 0, :], start=True, stop=False)
                        nTe.matmul(zps[:, jj, :], hz2[1][:, ts(c0 + jj, 128)],
                                   w["pz3"][:, 1, :], start=False, stop=True)
                    # ls = clip(raw + b3s, -5, 2); qls = sum_d ls
                    lsa = sp.tile([128, NH, dL], F32, name="lsa")
                    nV.tensor_tensor(out=lsa, in0=zps[:, :, dL:2 * dL],
                                     in1=w["b3s_rep"][:, 0:NH * dL].rearrange(
                                         "p (c d) -> p c d", c=NH),
                                     op=ALU.add)
                    nV.tensor_scalar(lsa, lsa, 2.0, -5.0,
                                     op0=ALU.min, op1=ALU.max)
                    nV.tensor_reduce(out=qls[:, c0:c0 + NH], in_=lsa,
                                     axis=AX.X, op=ALU.add)
                    els = sp.tile([128, NH, dL], F32, name="els")
                    nS.activation(out=els, in_=lsa, func=AF.Exp)
                    p1 = sp.tile([128, NH, dL], F32, name="p1")
                    nV.tensor_mul(p1, els, eps_t[:, c0:c0 + NH, :])
                    nV.tensor_tensor(out=znew16[:, c0:c0 + NH, :], in0=p1,
                                     in1=zps[:, :, 0:dL], op=ALU.add)

            # ---------------- znT transpose ----------------
            znT = sp.tile([dL, NT], F16, name="znT")
            for c in range(NCB):
                tps = psC.tile([128, 128], F16, tag="ps", name="ztp")
                nTe.transpose(tps, znew16[:, c, :], ident16)
                if c % 2 == 0:
                    nV.tensor_copy(out=znT[:, ts(c, 128)], in_=tps)
                else:
                    nS.activation(out=znT[:, ts(c, 128)], in_=tps, func=AF.Copy)

            # ---------------- observation net ----------------
            oeh = []
            for m in range(2):
                ht = sp.tile([128, NT], F16, name="oeh_" + str(m))
                for b in range(B2):
                    pso = psA.tile([128, N], F32, tag="ps", name="o1")
                    nTe.matmul(pso, w["oe1_z"][:, ts(m, 128)], znT[:, ts(b, N)],
                               start=True, stop=True)
                    nS.activation(out=ht[:, ts(b, N)], in_=pso, func=AF.Silu,
                                  bias=l1bias(2, m, b))
                oeh.append(ht)
            em2 = sp.tile([128, NT], F16, name="em2")
            for b in range(B2):
                pso = psA.tile([128, N], F32, tag="ps", name="o2")
                nTe.matmul(pso, w["oe2"][:, 0, :], oeh[0][:, ts(b, N)],
                           start=True, stop=False)
                nTe.matmul(pso, w["oe2"][:, 1, :], oeh[1][:, ts(b, N)],
                           start=False, stop=True)
                nS.activation(out=em2[:, ts(b, N)], in_=pso, func=AF.Silu,
                              bias=w["b2o"][:, 0:1])
            o3ps = psC.tile([128, NCB, 2], F32, tag="ps", name="o3")
            for c in range(NCB):
                nTe.matmul(o3ps[:, c, :], em2[:, ts(c, 128)], w["oe3"],
                           start=True, stop=True)

            # ---------------- pr layer-2 (flip) ----------------
            r2ps = psC.tile([128, NCB, Ka], F32, tag="ps", name="pr2")
            for c in range(NCB):
                nTe.matmul(r2ps[:, c, :], prh[0][:, ts(c, 128)], w["pr2"][:, 0, :],
                           start=True, stop=False)
                nTe.matmul(r2ps[:, c, :], prh[1][:, ts(c, 128)], w["pr2"][:, 1, :],
                           start=False, stop=True)
            rlog = sp.tile([128, NCB, Ka], F32, name="rlog")
            nV.tensor_tensor(out=rlog, in0=r2ps,
                             in1=w["prb2_rep"][:, :].rearrange(
                                 "p (c k) -> p c k", c=NCB), op=ALU.add)

            # ======= tail (natural_log_exp table set from here) =======
            ern = sp.tile([128, NCB, Ka], F32, name="ern")
            nS.activation(out=ern, in_=rlog, func=AF.Exp)
            u_t = sp.tile([128, NCB], F32, name="u_t")
            nS.activation(out=u_t, in_=o3ps[:, :, 1], func=AF.Exp,
                          bias=w["oe3b1"][:, 0:1])
            spv = sp.tile([128, NCB], F32, name="spv")
            nS.activation(out=spv, in_=u_t, func=AF.Ln, bias=1.0)  # softplus

            dn = sp.tile([128, NCB], F32, name="dn")
            nV.tensor_reduce(out=dn, in_=ern, axis=AX.X, op=ALU.add)
            nV.tensor_scalar_add(dn, dn, float(Kt - Ka))
            rdn = sp.tile([128, NCB], F32, name="rdn")
            nV.reciprocal(out=rdn, in_=dn)
            smu = sp.tile([128, NCB, Ka], F32, name="smu")
            nV.tensor_tensor(out=smu, in0=ern,
                             in1=w["scales_rep"][:, :].rearrange(
                                 "p (c k) -> p c k", c=NCB), op=ALU.mult)
            smult = sp.tile([128, NCB], F32, name="smult")
            nV.tensor_reduce(out=smult, in_=smu, axis=AX.X, op=ALU.add)
            nV.tensor_mul(smult, smult, rdn)

            sig = sp.tile([128, NCB], F32, name="sig")
            nV.tensor_mul(sig, spv, smult)
            nV.tensor_scalar(sig, sig, 5.0, 0.1, op0=ALU.min, op1=ALU.max)
            rsig = sp.tile([128, NCB], F32, name="rsig")
            nV.reciprocal(out=rsig, in_=sig)
            dev = sp.tile([128, NCB], F32, name="dev")
            for b in range(B2):
                nV.tensor_scalar_sub(dev[:, b * NC:(b + 1) * NC],
                                     o3ps[:, b * NC:(b + 1) * NC, 0],
                                     hb_t[:, 12 + b:13 + b])
            nV.tensor_mul(dev, dev, rsig)
            sq = sp.tile([128, NCB], F32, name="sq")
            nV.tensor_mul(sq, dev, dev)
            qtot = sp.tile([128, NCB], F32, name="qtot")
            nV.tensor_tensor(out=qtot, in0=qls, in1=hb_t[:, 14:22], op=ALU.add)
            # lw = lwn - ln(sig): keep the ln implicit (exp(-ln sig) = rsig).
            # M' = max(lwn) + ln(5) >= max(lw) since -ln(sig) <= ln(1/0.1).
            lwn = sp.tile([128, NCB], F32, name="lwn")
            nV.scalar_tensor_tensor(out=lwn, in0=sq, scalar=-0.5, in1=qtot,
                                    op0=ALU.mult, op1=ALU.add)

            mxc = sp.tile([128, B2], F32, name="mxc")
            for b in range(B2):
                nV.tensor_reduce(out=mxc[:, b:b + 1],
                                 in_=lwn[:, b * NC:(b + 1) * NC],
                                 axis=AX.X, op=ALU.max)
            mrow = sp.tile([1, B2], F32, name="mrow")
            nG.tensor_reduce(out=mrow, in_=mxc, axis=AX.C, op=ALU.max)
            nG.tensor_scalar(mrow, mrow, -1.0, -2.302586, op0=ALU.mult,
                             op1=ALU.add)
            nmb_ps = psC.tile([128, B2], F32, tag="ps", name="nmb_ps")
            nTe.matmul(nmb_ps, ones32, mrow, start=True, stop=True)
            nmb = sp.tile([128, B2], F32, name="nmb")
            nV.tensor_copy(out=nmb, in_=nmb_ps)
            lwm = sp.tile([128, NCB], F32, name="lwm")
            for b in range(B2):
                nV.tensor_scalar_add(lwm[:, b * NC:(b + 1) * NC],
                                     lwn[:, b * NC:(b + 1) * NC],
                                     nmb[:, b:b + 1])
            ewp = sp.tile([128, NCB], F32, name="ewp")
            nS.activation(out=ewp, in_=lwm, func=AF.Exp)
            e_w = sp.tile([128, NCB], F32, name="e_w")
            nV.tensor_mul(e_w, ewp, rsig)
            ew16 = sp.tile([128, NCB], F16, name="ew16")
            nV.tensor_copy(out=ew16, in_=e_w)

            # weighted-mean output
            sw = sp.tile([128, B2], F32, name="sw")
            for b in range(B2):
                nV.tensor_reduce(out=sw[:, b:b + 1],
                                 in_=e_w[:, b * NC:(b + 1) * NC],
                                 axis=AX.X, op=ALU.add)
            swrow = sp.tile([1, B2], F32, name="swrow")
            nG.tensor_reduce(out=swrow, in_=sw, axis=AX.C, op=ALU.add)
            swa_ps = psC.tile([128, B2], F32, tag="ps", name="swa_ps")
            nTe.matmul(swa_ps, ones32, swrow, start=True, stop=True)
            rse = sp.tile([128, B2], F32, name="rse")
            nV.reciprocal(out=rse, in_=swa_ps)
            for b in range(B2):
                mz = psC.tile([128, 1], F32, tag="ps", name="mz")
                for jc in range(NC):
                    c = b * NC + jc
                    nTe.matmul(mz, znew16[:, c, :], ew16[:, c:c + 1],
                               start=(jc == 0), stop=(jc == NC - 1))
                nV.scalar_tensor_tensor(out=means_acc[:, b, t:t + 1], in0=mz,
                                        scalar=rse[:, b:b + 1],
                                        in1=w["b3m_col"],
                                        op0=ALU.mult, op1=ALU.add)

            # ---------------- soft resample (skipped on last step) --------
            if last:
                continue
            sj = sp.tile([128, NCB], F32, name="sj")
            nV.tensor_mul(sj, e_w, e_w)  # exp(2*(lw-M))
            zs = sp.tile([128, NCB, dL], BF16, name="zs")
            rl9s = sp.tile([128, NCB, Ka + 1], BF16, name="rl9s")
            nV.tensor_copy(out=rl9s[:, :, 0], in_=sj)
            g1s, g2s = [], []
            for b in range(B2):
                # per-b prep on gpsimd (sbuf-only) so PE can start b0's
                # matmuls while b1's prep still runs
                for jc in range(NC):
                    c = b * NC + jc
                    nV.tensor_scalar_mul(zs[:, c, :], znew16[:, c, :],
                                         sj[:, c:c + 1])
                    nV.tensor_scalar_mul(rl9s[:, c, 1:Ka + 1], rlog[:, c, :],
                                         sj[:, c:c + 1])
                g1 = psA.tile([dL, N], F32, tag="ps", name="g1")
                g2 = psA.tile([Ka + 1, N], F32, tag="ps", name="g2")
                for jc in range(NC):
                    c = b * NC + jc
                    nTe.matmul(g1, zs[:, c, :], P_t[:, c, :],
                               start=(jc == 0), stop=(jc == NC - 1))
                    nTe.matmul(g2, rl9s[:, c, :], P_t[:, c, :],
                               start=(jc == 0), stop=(jc == NC - 1))
                g1s.append(g1)
                g2s.append(g2)
                # 1/D row via Ln -> Exp(-1) on ACT (ln_exp set, no switch);
                # f32r PE broadcast (1 cyc/row), one psum->sbuf copy.
                dln = sp.tile([1, N], F32, name="dln")
                nS.activation(out=dln, in_=g2[0:1, :], func=AF.Ln)
                rdr = sp.tile([1, N], F32R, name="rdr")
                nS.activation(out=rdr, in_=dln, func=AF.Exp, scale=-1.0)
                rdbc = psA.tile([128, N], F32, tag="ps", name="rdbc")
                nTe.matmul(rdbc, ones_r, rdr, start=True, stop=True)
                rdsb = sp.tile([128, N], F32, name="rdsb")
                nV.tensor_copy(out=rdsb, in_=rdbc)
                nV.tensor_mul(zT[:, ts(b, N)], g1, rdsb)
                nV.tensor_mul(rlr_rows[:, ts(b, N)], g2,
                              rdsb[0:Ka + 1, :])

        # ---------------- write outputs ----------------
        for b in range(B2):
            nY.dma_start(out=d_out[b], in_=means_acc[:, b, :])

    return split_waits(nc)


# ======================= host side =======================

def _f16(x):
    return np.asarray(x, np.float32).astype(np.float16)


def _bf16(x):
    return np.asarray(x, np.float32).astype(ml_dtypes.bfloat16)


def _rep_row(row, p=128):
    """replicate a row vector across 128 partitions."""
    r = np.asarray(row, np.float32).reshape(-1)
    return np.broadcast_to(r[None, :], (p, r.shape[0])).copy()


def host_prep(inputs, t_steps=T_FULL):
    obs = np.asarray(inputs["obs"], np.float32)[:t_steps]
    h_seq = np.asarray(inputs["h_seq"], np.float32)[:t_steps]
    z0 = np.asarray(inputs["z0"], np.float32)
    rl0 = np.asarray(inputs["regime_logits0"], np.float32)
    eps = np.asarray(inputs["eps"], np.float32)[:t_steps]
    u = np.asarray(inputs["gumbel_u"], np.float32)[:t_steps]
    assert int(inputs["k_active"]) == Ka

    pz_w1 = np.asarray(inputs["pz_w1"], np.float32)
    pr_w1 = np.asarray(inputs["pr_w1"], np.float32)
    oe_w1 = np.asarray(inputs["oe_w1"], np.float32)
    pz_b1 = np.asarray(inputs["pz_b1"], np.float32)
    pr_b1 = np.asarray(inputs["pr_b1"], np.float32)
    oe_b1 = np.asarray(inputs["oe_b1"], np.float32)
    pz_b3 = np.asarray(inputs["pz_b3"], np.float32)
    oe_b3 = np.asarray(inputs["oe_b3"], np.float32)
    emb_a = np.asarray(inputs["pe_emb"], np.float32)[:Ka]
    b3m = pz_b3[:dL]

    # exp(g/TEMP) = x^-2  with x = -log(u+1e-10)+1e-10 (TEMP=0.5)
    x = (-np.log(u + np.float32(1e-10)) + np.float32(1e-10)).astype(np.float32)
    P = (1.0 / (x * x)).astype(np.float32)

    # t=0 regime softmax (active slice), rows layout
    e0 = np.exp(rl0 - rl0.max(axis=-1, keepdims=True))
    rp0 = (e0 / e0.sum(axis=-1, keepdims=True))[:, :, :Ka]  # (B,N,8)

    # per-(t,b) layer-1 bias pack + y' + qeps
    # l1 input rows: [h (dM), z (dL), e (dE)]
    qeps_const = (dL - 1) * 0.5 * LOG2PI

    def l1_bias(w1, b1v):
        bias_t = np.einsum('tbm,mh->tbh', h_seq, w1[:dM]) + b1v
        bias_t[1:] += w1[dM:dM + dL].T @ b3m   # carry excludes b3m for t>=1
        return bias_t.reshape(t_steps, B, 2, 128)  # (t, B, m, 128)

    bias_pz = l1_bias(pz_w1, pz_b1)
    bias_pr = l1_bias(pr_w1, pr_b1)
    # oe bias: h part rows oe_w1[dL:], + oe1_z^T b3m always
    bias_oe = np.einsum('tbm,mh->tbh', h_seq, oe_w1[dL:]) + oe_b1
    bias_oe += oe_w1[:dL].T @ b3m
    bias_oe = bias_oe.reshape(t_steps, B, 2, 128)

    yprime = obs - oe_b3[0]                     # (t, B)
    qeps = 0.5 * (eps.astype(np.float32) ** 2).sum(-1) + qeps_const  # (t,B,N)

    wmap = {
        "pz1_z": _f16(pz_w1[dM:dM + dL]),
        "pr1_z": _f16(pr_w1[dM:dM + dL]),
        "oe1_z": _f16(oe_w1[:dL]),
        "vpz": _f16(emb_a @ pz_w1[dM + dL:]),
        "vpr": _f16(emb_a @ pr_w1[dM + dL:]),
        "pz2": _f16(inputs["pz_w2"]), "pz3": _f16(inputs["pz_w3"]),
        "pr2": _f16(inputs["pr_w2"]),
        "oe2": _f16(inputs["oe_w2"]), "oe3": _f16(inputs["oe_w3"]),
        "b2z": np.asarray(inputs["pz_b2"], np.float32).reshape(2, 128).T.copy(),
        "b2o": np.asarray(inputs["oe_b2"], np.float32).reshape(1, 128).T.copy(),
        "b3s_rep": np.tile(_rep_row(pz_b3[dL:]), (1, NC)),
        "b3m_col": np.repeat(b3m[:, None], 1, axis=1).astype(np.float32),
        "oe3b1": np.full((128, 1), oe_b3[1], np.float32),
        "scales_rep": np.tile(_rep_row(np.log1p(np.exp(
            np.asarray(inputs["log_obs_scale"], np.float32)[:Ka]))), (1, NCB)),
        "prb2_rep": np.tile(_rep_row(np.asarray(inputs["pr_b2"], np.float32)),
                            (1, NCB)),
    }

    in_maps = []
    for core in range(NCORES):
        bp = [2 * core, 2 * core + 1]
        m = dict(wmap)
        m["pT"] = _bf16(P[:, bp].transpose(0, 1, 3, 2))       # (T,2,j,i)
        m["eps_n"] = _f16(eps[:, bp])                          # (T,2,N,dL)
        m["z0T"] = _f16(np.concatenate([z0[b].T for b in bp], axis=1))
        m["rp0T"] = _f16(np.concatenate([rp0[b].T for b in bp], axis=1))
        hbc = np.zeros((t_steps, 128, NHB), np.float32)
        for bi, bb in enumerate(bp):
            for mm in range(2):
                hbc[:, :, 0 * 4 + mm * 2 + bi] = bias_pz[:, bb, mm]
                hbc[:, :, 1 * 4 + mm * 2 + bi] = bias_pr[:, bb, mm]
                hbc[:, :, 2 * 4 + mm * 2 + bi] = bias_oe[:, bb, mm]
            hbc[:, :, 12 + bi] = yprime[:, bb, None]
            q = qeps[:, bb].reshape(t_steps, NC, 128)          # (t, jc, j)
            for jc in range(NC):
                hbc[:, :, 14 + bi * NC + jc] = q[:, jc]
        m["hb"] = hbc
        in_maps.append(m)
    return in_maps


def gather_output(results, t_steps=T_FULL):
    out = np.zeros((t_steps, B, dL), np.float32)
    for core in range(NCORES):
        r = results[core]["means"]                             # (2,128,T)
        for b in range(B2):
            out[:, 2 * core + b, :] = np.asarray(r[b], np.float32).T
    return out


def kernel(**inputs):
    from concourse.bass_utils import run_bass_kernel_spmd
    nc = build_core_program(T_FULL)
    in_maps = host_prep(inputs, T_FULL)
    res = run_bass_kernel_spmd(nc, in_maps, list(range(NCORES)))
    return gather_output(res.results, T_FULL)


if __name__ == "__main__":
    nc = build_core_program(2)
    print("built ok")


# revision 64
# speedup vs baseline: 1.2370x; 1.0075x over previous
"""Differentiable particle filter V3 — Trainium2 Bass kernel (optimized).

Data-parallel over batch B=16 across 8 NeuronCores (2 batch items/core).
Each core runs the T=16 sequential scan for its two particle clouds
(N=512, dL=128) with activations on-chip, feature-on-partition /
particle-on-free layout so MLP layers are PE matmul chains.

Optimizations vs v1:
- Host precomputes: per-(t,b) layer-1 biases (h contraction + b3m fold),
  y' = y - oe_b3[0], 0.5*sum(eps^2)+const, t=0 regime softmax, and
  V = emb @ W_e folding (kills remb matmuls and all on-device h work).
- 2 ACT table-set switches/step: exp(log_std) via tanh identity
  (tanh lives in the silu table set), whole tail uses natural_log_exp set.
- s_j folded into resample lhs (znew_s, rl9s); denominator rides g2 as a
  9th lhs column; 1/D broadcast via K=1 PE matmul (no DRAM bounce).
- Cross-partition max/sum via gpsimd.partition_all_reduce.
- 3 DMA descriptors per step issued from the idle sync engine.
- Resample skipped on the final step (carry unused).
"""

import numpy as np
import ml_dtypes

import concourse.bass as bass
import concourse.tile as tile
from concourse import mybir
from concourse import bass_isa
from concourse.masks import make_identity
from concourse.bass import ts

F32 = mybir.dt.float32
F32R = mybir.dt.float32r
F16 = mybir.dt.float16
BF16 = mybir.dt.bfloat16
AF = mybir.ActivationFunctionType
ALU = mybir.AluOpType
AX = mybir.AxisListType
RED = bass_isa.ReduceOp

LOG2PI = 1.8378770664093453

# problem dims (hardcoded per spec)
B, N, T_FULL = 16, 512, 16
dL, dM, dE, H = 128, 256, 32, 256
Kt, Ka = 18, 8
NCORES = 8
B2 = 2          # batch items per core
NC = 4          # 128-particle chunks per batch item
NCB = NC * B2   # particle chunks per core (8)
NT = B2 * N     # particles per core (1024)
NHB = 22        # host bias pack columns: 12 l1-bias, 2 y', 8 qeps


def split_waits(nc, limit=1):
    """This walrus build encodes at most one sync wait per instruction.
    Hoist excess waits onto injected same-engine NOPs placed immediately
    before the instruction (engine program order preserves semantics)."""
    for f in nc.m.functions:
        for bb in f.blocks:
            newl = []
            for ins in bb.instructions:
                si = ins.sync_info
                if si is not None and si.on_wait and len(si.on_wait) > limit:
                    waits = list(si.on_wait)
                    for k, wv in enumerate(waits[:-limit]):
                        nop = mybir.InstNoOp(
                            name=f"{ins.name}-ws{k}", ins=[], outs=[])
                        nop.engine = ins.engine
                        nop.sync_info = mybir.SyncInfo(on_wait=[wv], on_update=[])
                        newl.append(nop)
                    si.on_wait = waits[-limit:]
                newl.append(ins)
            try:
                bb.instructions = newl
            except Exception:
                bb.instructions.clear()
                bb.instructions.extend(newl)
    return nc


def build_core_program(t_steps=T_FULL):
    nc = bass.Bass()

    # ---------------- DRAM parameters (per-core shapes) ----------------
    d_pT = nc.declare_dram_parameter("pT", [t_steps, B2, N, N], BF16, isOutput=False)
    d_eps = nc.declare_dram_parameter("eps_n", [t_steps, B2, N, dL], F16, isOutput=False)
    d_hb = nc.declare_dram_parameter("hb", [t_steps, 128, NHB], F32, isOutput=False)
    d_z0T = nc.declare_dram_parameter("z0T", [dL, NT], F16, isOutput=False)
    d_rp0 = nc.declare_dram_parameter("rp0T", [Ka, NT], F16, isOutput=False)

    d_w = {}
    for name, shape, dt in [
        ("pz1_z", [dL, H], F16), ("pr1_z", [dL, H], F16), ("oe1_z", [dL, H], F16),
        ("vpz", [Ka, H], F16), ("vpr", [Ka, H], F16),
        ("pz2", [H, H], F16), ("pz3", [H, 2 * dL], F16),
        ("pr2", [H, Ka], F16),
        ("oe2", [H, 128], F16), ("oe3", [128, 2], F16),
        ("b2z", [128, 2], F32),          # pz_b2 cols
        ("b2o", [128, 1], F32),          # oe_b2 col
        ("b3s_rep", [128, NC * dL], F32),  # pz_b3[dL:] row replicated+tiled 4x
        ("b3m_col", [128, 1], F32),      # pz_b3[:dL] as partition col
        ("oe3b1", [128, 1], F32),        # oe_b3[1] replicated col
        ("scales_rep", [128, NCB * Ka], F32),  # softplus(scales) tiled 8x
        ("prb2_rep", [128, NCB * Ka], F32),    # pr_b2 tiled 8x
    ]:
        d_w[name] = nc.declare_dram_parameter(name, shape, dt, isOutput=False)

    d_out = nc.declare_dram_parameter("means", [B2, dL, t_steps], F32, isOutput=True)

    from contextlib import ExitStack
    with tile.TileContext(nc) as tc, ExitStack() as ctx:
        wp = ctx.enter_context(tc.tile_pool(name="wp", bufs=1))
        sp = ctx.enter_context(tc.tile_pool(name="sp", bufs=2))
        pp = ctx.enter_context(tc.tile_pool(name="pp", bufs=2))
        st = ctx.enter_context(tc.tile_pool(name="st", bufs=1))
        psA = ctx.enter_context(tc.tile_pool(name="psA", bufs=4, space="PSUM"))
        psC = ctx.enter_context(tc.tile_pool(name="psC", bufs=4, space="PSUM"))

        nV, nS, nG, nTe, nY = nc.vector, nc.scalar, nc.gpsimd, nc.tensor, nc.sync

        # ---------------- load weights / constants ----------------
        w = {}
        for wi, (name, h) in enumerate(d_w.items()):
            eng = nG if wi % 2 == 0 else nY
            shp = list(h.shape)
            if shp[0] > 128:
                assert shp[0] == 256
                tl = wp.tile([128, 2, shp[1]], h.dtype, name="w_" + name)
                eng.dma_start(out=tl, in_=h[:, :].rearrange("(c p) m -> p c m", p=128))
            else:
                tl = wp.tile(shp, h.dtype, name="w_" + name)
                eng.dma_start(out=tl, in_=h[:, :])
            w[name] = tl

        ident16 = wp.tile([128, 128], F16, name="ident16")
        make_identity(nc, ident16)
        ident32 = wp.tile([128, 128], F32, name="ident32")
        make_identity(nc, ident32)
        ones16 = wp.tile([1, 128], F16, name="ones16")
        nV.memset(ones16, 1.0)
        ones32 = wp.tile([1, 128], F32, name="ones32")
        nV.memset(ones32, 1.0)
        ones_bf = wp.tile([1, 128], BF16, name="ones_bf")
        nV.memset(ones_bf, 1.0)
        ones_r = wp.tile([1, 128], F32R, name="ones_r")
        nV.tensor_copy(out=ones_r, in_=ones32)


        # ---------------- state ----------------
        zT = st.tile([dL, NT], F16, name="zT_state")
        nG.dma_start(out=zT, in_=d_z0T[:, :])
        rp0 = st.tile([Ka, NT], F16, name="rp0_state")
        nG.dma_start(out=rp0, in_=d_rp0[:, :])
        # row 0 = D/D = 1 (ignored), rows 1..8 = normalized regime logits
        rlr_rows = st.tile([Ka + 1, NT], F32, name="rlr_rows")
        means_acc = st.tile([dL, B2, t_steps], F32, name="means_acc")

        # Collapse the weight/state-load DMA deps into one barrier (see
        # split_waits): a chain of tiny DVE reads accumulates every DMA
        # tick into DVE's vector clock; one NOP then covers all loads.
        from concourse.tile import add_dep_helper
        probe = st.tile([1, 1], F32, name="probe")
        last_copy = None
        for tl in [*w.values(), ident16, zT, rp0]:
            src = tl[0:1, 0, 0:1] if len(tl.shape) == 3 else tl[0:1, 0:1]
            last_copy = nV.tensor_copy(out=probe, in_=src)
        curr_bb = nc.cur_bb
        bar = nc.sync.nop()
        assert last_copy is not None
        add_dep_helper(bar.ins, last_copy.ins, sync=True, reason="weights barrier")
        tc.barrier_instruction_and_bb = (bar.ins, curr_bb)

        for t in range(t_steps):
            last = (t == t_steps - 1)
            # ---------------- step input DMAs (sync engine) ----------------
            if not last:
                P_t = pp.tile([128, NCB, N], BF16, name="P_t")
                nY.dma_start(out=P_t,
                             in_=d_pT[t].rearrange("b (c p) i -> p (b c) i", p=128))
            eps_t = sp.tile([128, NCB, dL], F16, name="eps_t")
            nY.dma_start(out=eps_t,
                         in_=d_eps[t].rearrange("b (c p) d -> p (b c) d", p=128))
            hb_t = sp.tile([128, NHB], F32, name="hb_t")
            nY.dma_start(out=hb_t, in_=d_hb[t])

            def l1bias(net, m, b):
                c = net * 4 + m * 2 + b
                return hb_t[:, c:c + 1]

            # pz layer-1 z-passes first: PE chews on these (psum left open,
            # start w/o stop) while the rl-softmax block runs on DVE/ACT.
            pz_zps = []
            for m in range(2):
                row = []
                for b in range(B2):
                    ps1 = psA.tile([128, N], F32, tag="ps", name="l1")
                    nTe.matmul(ps1, w["pz1_z"][:, ts(m, 128)], zT[:, ts(b, N)],
                               start=True, stop=False)
                    row.append(ps1)
                pz_zps.append(row)

            # ---------------- regime softmax -> rp_rows (8, NT) f16 -------
            # ACT exp here rides the ln_exp table set left over from the
            # previous step's tail (no switch).
            if t == 0:
                rp_rows = rp0
            else:
                rp_rows = sp.tile([Ka, NT], F16, name="rp_rows")
                rlc = sp.tile([128, NCB, Ka + 1], F32, name="rlc")
                for c in range(NCB):
                    tp = psC.tile([128, Ka + 1], F32, tag="ps", name="rltp")
                    nTe.transpose(tp, rlr_rows[:, ts(c, 128)],
                                  ident32[0:Ka + 1, 0:Ka + 1])
                    if c % 2 == 0:
                        nV.tensor_copy(out=rlc[:, c, :], in_=tp)
                    else:
                        nS.activation(out=rlc[:, c, :], in_=tp, func=AF.Copy)
                ernr = sp.tile([128, NCB, Ka], F32, name="ernr")
                nS.activation(out=ernr, in_=rlc[:, :, 1:Ka + 1], func=AF.Exp)
                dnr = sp.tile([128, NCB], F32, name="dnr")
                nV.tensor_reduce(out=dnr, in_=ernr, axis=AX.X, op=ALU.add)
                nV.tensor_scalar_add(dnr, dnr, float(Kt - Ka))
                rdnr = sp.tile([128, NCB], F32, name="rdnr")
                nV.reciprocal(out=rdnr, in_=dnr)
                rpc = sp.tile([128, NCB, Ka], F16, name="rpc")
                for c in range(NCB):
                    nV.tensor_scalar_mul(rpc[:, c, :], ernr[:, c, :],
                                         rdnr[:, c:c + 1])
                for c in range(NCB):
                    tpb = psC.tile([Ka, 128], F16, tag="ps", name="rptp")
                    nTe.transpose(tpb, rpc[:, c, :], ident16)
                    if c % 2 == 0:
                        nV.tensor_copy(out=rp_rows[:, ts(c, 128)], in_=tpb)
                    else:
                        nS.activation(out=rp_rows[:, ts(c, 128)], in_=tpb,
                                      func=AF.Copy)

            # ---------------- layer 1 (pz, pr) ----------------
            # z-passes for pz were already issued before the rl-softmax
            # block (PE overlap); close them with the e-pass then silu.
            def layer1(wz, ve, net, nm, zps=None):
                out = []
                for m in range(2):
                    ht = sp.tile([128, NT], F16, name=nm + str(m))
                    for b in range(B2):
                        if zps is None:
                            ps1 = psA.tile([128, N], F32, tag="ps", name="l1")
                            nTe.matmul(ps1, wz[:, ts(m, 128)], zT[:, ts(b, N)],
                                       start=True, stop=False)
                        else:
                            ps1 = zps[m][b]
                        nTe.matmul(ps1, ve[:, ts(m, 128)], rp_rows[:, ts(b, N)],
                                   start=False, stop=True)
                        nS.activation(out=ht[:, ts(b, N)], in_=ps1, func=AF.Silu,
                                      bias=l1bias(net, m, b))
                    out.append(ht)
                return out

            hz1 = layer1(w["pz1_z"], w["vpz"], 0, "hz1_", zps=pz_zps)
            prh = layer1(w["pr1_z"], w["vpr"], 1, "prh_")

            # ---------------- layer 2 (pz2) ----------------
            hz2 = []
            for m in range(2):
                ht = sp.tile([128, NT], F16, name="hz2_" + str(m))
                for b in range(B2):
                    ps2 = psA.tile([128, N], F32, tag="ps", name="l2")
                    nTe.matmul(ps2, w["pz2"][:, 0, ts(m, 128)], hz1[0][:, ts(b, N)],
                               start=True, stop=False)
                    nTe.matmul(ps2, w["pz2"][:, 1, ts(m, 128)], hz1[1][:, ts(b, N)],
                               start=False, stop=True)
                    nS.activation(out=ht[:, ts(b, N)], in_=ps2, func=AF.Silu,
                                  bias=w["b2z"][:, m:m + 1])
                hz2.append(ht)

            # ---------------- pz3 (flip) + znew + log-q pieces -------------
            znew16 = sp.tile([128, NCB, dL], F16, name="znew16")
            qls = sp.tile([128, NCB], F32, name="qls")
            NH = 2  # chunks per pz3 psum tile
            for b in range(B2):
                for h2 in range(NC // NH):
                    c0 = b * NC + h2 * NH
                    zps = psA.tile([128, NH, 2 * dL], F32, tag="ps", name="zp")
                    for jj in range(NH):
                        nTe.matmul(zps[:, jj, :], hz2[0][:, ts(c0 + jj, 128)],
                                   w["pz3"][:, 0, :], start=True, stop=False)
                        nTe.matmul(zps[:, jj, :], hz2[1][:, ts(c0 + jj, 128)],
                                   w["pz3"][:, 1, :], start=False, stop=True)
                    # ls = clip(raw + b3s, -5, 2); qls = sum_d ls
                    lsa = sp.tile([128, NH, dL], F32, name="lsa")
                    nV.tensor_tensor(out=lsa, in0=zps[:, :, dL:2 * dL],
                                     in1=w["b3s_rep"][:, 0:NH * dL].rearrange(
                                         "p (c d) -> p c d", c=NH),
                                     op=ALU.add)
                    nV.tensor_scalar(lsa, lsa, 2.0, -5.0,
                                     op0=ALU.min, op1=ALU.max)
                    nV.tensor_reduce(out=qls[:, c0:c0 + NH], in_=lsa,
                                     axis=AX.X, op=ALU.add)
                    els = sp.tile([128, NH, dL], F32, name="els")
                    nS.activation(out=els, in_=lsa, func=AF.Exp)
                    p1 = sp.tile([128, NH, dL], F32, name="p1")
                    nV.tensor_mul(p1, els, eps_t[:, c0:c0 + NH, :])
                    nV.tensor_tensor(out=znew16[:, c0:c0 + NH, :], in0=p1,
                                     in1=zps[:, :, 0:dL], op=ALU.add)

            # ---------------- znT transpose ----------------
            znT = sp.tile([dL, NT], F16, name="znT")
            for c in range(NCB):
                tps = psC.tile([128, 128], F16, tag="ps", name="ztp")
                nTe.transpose(tps, znew16[:, c, :], ident16)
                if c % 2 == 0:
                    nV.tensor_copy(out=znT[:, ts(c, 128)], in_=tps)
                else:
                    nS.activation(out=znT[:, ts(c, 128)], in_=tps, func=AF.Copy)

            # ---------------- observation net ----------------
            oeh = []
            for m in range(2):
                ht = sp.tile([128, NT], F16, name="oeh_" + str(m))
                for b in range(B2):
                    pso = psA.tile([128, N], F32, tag="ps", name="o1")
                    nTe.matmul(pso, w["oe1_z"][:, ts(m, 128)], znT[:, ts(b, N)],
                               start=True, stop=True)
                    nS.activation(out=ht[:, ts(b, N)], in_=pso, func=AF.Silu,
                                  bias=l1bias(2, m, b))
                oeh.append(ht)
            em2 = sp.tile([128, NT], F16, name="em2")
            for b in range(B2):
                pso = psA.tile([128, N], F32, tag="ps", name="o2")
                nTe.matmul(pso, w["oe2"][:, 0, :], oeh[0][:, ts(b, N)],
                           start=True, stop=False)
                nTe.matmul(pso, w["oe2"][:, 1, :], oeh[1][:, ts(b, N)],
                           start=False, stop=True)
                nS.activation(out=em2[:, ts(b, N)], in_=pso, func=AF.Silu,
                              bias=w["b2o"][:, 0:1])
            o3ps = psC.tile([128, NCB, 2], F32, tag="ps", name="o3")
            for c in range(NCB):
                nTe.matmul(o3ps[:, c, :], em2[:, ts(c, 128)], w["oe3"],
                           start=True, stop=True)

            # ---------------- pr layer-2 (flip) ----------------
            r2ps = psC.tile([128, NCB, Ka], F32, tag="ps", name="pr2")
            for c in range(NCB):
                nTe.matmul(r2ps[:, c, :], prh[0][:, ts(c, 128)], w["pr2"][:, 0, :],
                           start=True, stop=False)
                nTe.matmul(r2ps[:, c, :], prh[1][:, ts(c, 128)], w["pr2"][:, 1, :],
                           start=False, stop=True)
            rlog = sp.tile([128, NCB, Ka], F32, name="rlog")
            nV.tensor_tensor(out=rlog, in0=r2ps,
                             in1=w["prb2_rep"][:, :].rearrange(
                                 "p (c k) -> p c k", c=NCB), op=ALU.add)

            # ======= tail (natural_log_exp table set from here) =======
            ern = sp.tile([128, NCB, Ka], F32, name="ern")
            nS.activation(out=ern, in_=rlog, func=AF.Exp)
            u_t = sp.tile([128, NCB], F32, name="u_t")
            nS.activation(out=u_t, in_=o3ps[:, :, 1], func=AF.Exp,
                          bias=w["oe3b1"][:, 0:1])
            spv = sp.tile([128, NCB], F32, name="spv")
            nS.activation(out=spv, in_=u_t, func=AF.Ln, bias=1.0)  # softplus

            dn = sp.tile([128, NCB], F32, name="dn")
            nV.tensor_reduce(out=dn, in_=ern, axis=AX.X, op=ALU.add)
            nV.tensor_scalar_add(dn, dn, float(Kt - Ka))
            rdn = sp.tile([128, NCB], F32, name="rdn")
            nV.reciprocal(out=rdn, in_=dn)
            smu = sp.tile([128, NCB, Ka], F32, name="smu")
            nV.tensor_tensor(out=smu, in0=ern,
                             in1=w["scales_rep"][:, :].rearrange(
                                 "p (c k) -> p c k", c=NCB), op=ALU.mult)
            smult = sp.tile([128, NCB], F32, name="smult")
            nV.tensor_reduce(out=smult, in_=smu, axis=AX.X, op=ALU.add)
            nV.tensor_mul(smult, smult, rdn)

            sig = sp.tile([128, NCB], F32, name="sig")
            nV.tensor_mul(sig, spv, smult)
            nV.tensor_scalar(sig, sig, 5.0, 0.1, op0=ALU.min, op1=ALU.max)
            rsig = sp.tile([128, NCB], F32, name="rsig")
            nV.reciprocal(out=rsig, in_=sig)
            dev = sp.tile([128, NCB], F32, name="dev")
            for b in range(B2):
                nV.tensor_scalar_sub(dev[:, b * NC:(b + 1) * NC],
                                     o3ps[:, b * NC:(b + 1) * NC, 0],
                                     hb_t[:, 12 + b:13 + b])
            nV.tensor_mul(dev, dev, rsig)
            sq = sp.tile([128, NCB], F32, name="sq")
            nV.tensor_mul(sq, dev, dev)
            qtot = sp.tile([128, NCB], F32, name="qtot")
            nV.tensor_tensor(out=qtot, in0=qls, in1=hb_t[:, 14:22], op=ALU.add)
            # lw = lwn - ln(sig): keep the ln implicit (exp(-ln sig) = rsig).
            # M' = max(lwn) + ln(5) >= max(lw) since -ln(sig) <= ln(1/0.1).
            lwn = sp.tile([128, NCB], F32, name="lwn")
            nV.scalar_tensor_tensor(out=lwn, in0=sq, scalar=-0.5, in1=qtot,
                                    op0=ALU.mult, op1=ALU.add)

            # ---------- per-b: max, weights, mean, resample ----------
            # fully per-b so b0's PE resample overlaps b1's weight chain
            lwm = sp.tile([128, NCB], F32, name="lwm")
            ewp = sp.tile([128, NCB], F32, name="ewp")
            e_w = sp.tile([128, NCB], F32, name="e_w")
            ew16 = sp.tile([128, NCB], F16, name="ew16")
            sj = sp.tile([128, NCB], F32, name="sj")
            zs = sp.tile([128, NCB, dL], BF16, name="zs")
            rl9s = sp.tile([128, NCB, Ka + 1], BF16, name="rl9s")
            g1s, g2s, rses = [], [], []
            for b in range(B2):
                bs = slice(b * NC, (b + 1) * NC)
                mxc = sp.tile([128, 1], F32, name="mxc")
                nV.tensor_reduce(out=mxc, in_=lwn[:, bs], axis=AX.X, op=ALU.max)
                mrow = sp.tile([1, 1], F32, name="mrow")
                nG.tensor_reduce(out=mrow, in_=mxc, axis=AX.C, op=ALU.max)
                nG.tensor_scalar(mrow, mrow, -1.0, -2.302586, op0=ALU.mult,
                                 op1=ALU.add)
                nmb_ps = psC.tile([128, 1], F32, tag="ps", name="nmb_ps")
                nTe.matmul(nmb_ps, ones32, mrow, start=True, stop=True)
                nmb = sp.tile([128, 1], F32, name="nmb")
                nV.tensor_copy(out=nmb, in_=nmb_ps)
                nV.tensor_scalar_add(lwm[:, bs], lwn[:, bs], nmb)
                nS.activation(out=ewp[:, bs], in_=lwm[:, bs], func=AF.Exp)
                nV.tensor_mul(e_w[:, bs], ewp[:, bs], rsig[:, bs])
                nV.tensor_copy(out=ew16[:, bs], in_=e_w[:, bs])
                # weighted-mean output
                sw = sp.tile([128, 1], F32, name="sw")
                nV.tensor_reduce(out=sw, in_=e_w[:, bs], axis=AX.X, op=ALU.add)
                swrow = sp.tile([1, 1], F32, name="swrow")
                nG.tensor_reduce(out=swrow, in_=sw, axis=AX.C, op=ALU.add)
                swa_ps = psC.tile([128, 1], F32, tag="ps", name="swa_ps")
                nTe.matmul(swa_ps, ones32, swrow, start=True, stop=True)
                rse = sp.tile([128, 1], F32, name="rse")
                nV.reciprocal(out=rse, in_=swa_ps)
                rses.append(rse)
                if last:
                    continue
                # resample prep for this b
                nV.tensor_mul(sj[:, bs], e_w[:, bs], e_w[:, bs])
                nV.tensor_copy(out=rl9s[:, bs, 0], in_=sj[:, bs])
                for jc in range(NC):
                    c = b * NC + jc
                    nV.tensor_scalar_mul(zs[:, c, :], znew16[:, c, :],
                                         sj[:, c:c + 1])
                    nV.tensor_scalar_mul(rl9s[:, c, 1:Ka + 1], rlog[:, c, :],
                                         sj[:, c:c + 1])
            if not last:
                for b in range(B2):
                    g1 = psA.tile([dL, N], F32, tag="ps", name="g1")
                    g2 = psA.tile([Ka + 1, N], F32, tag="ps", name="g2")
                    for jc in range(NC):
                        c = b * NC + jc
                        nTe.matmul(g1, zs[:, c, :], P_t[:, c, :],
                                   start=(jc == 0), stop=(jc == NC - 1))
                        nTe.matmul(g2, rl9s[:, c, :], P_t[:, c, :],
                                   start=(jc == 0), stop=(jc == NC - 1))
                    g1s.append(g1)
                    g2s.append(g2)
            for b in range(B2):
                # weighted-mean output (off the critical path)
                mz = psC.tile([128, 1], F32, tag="ps", name="mz")
                for jc in range(NC):
                    c = b * NC + jc
                    nTe.matmul(mz, znew16[:, c, :], ew16[:, c:c + 1],
                               start=(jc == 0), stop=(jc == NC - 1))
                nV.scalar_tensor_tensor(out=means_acc[:, b, t:t + 1], in0=mz,
                                        scalar=rses[b], in1=w["b3m_col"],
                                        op0=ALU.mult, op1=ALU.add)
                if last:
                    continue
                # 1/D row via Ln -> Exp(-1) on ACT (ln_exp set, no switch);
                # f32r PE broadcast (1 cyc/row), one psum->sbuf copy.
                dln = sp.tile([1, N], F32, name="dln")
                nS.activation(out=dln, in_=g2s[b][0:1, :], func=AF.Ln)
                rdr = sp.tile([1, N], F32R, name="rdr")
                nS.activation(out=rdr, in_=dln, func=AF.Exp, scale=-1.0)
                rdbc = psC.tile([128, N], F32, tag="ps", name="rdbc")
                nTe.matmul(rdbc, ones_r, rdr, start=True, stop=True)
                rdsb = sp.tile([128, N], F32, name="rdsb")
                nV.tensor_copy(out=rdsb, in_=rdbc)
                nV.tensor_mul(zT[:, ts(b, N)], g1s[b], rdsb)
                nV.tensor_mul(rlr_rows[:, ts(b, N)], g2s[b],
                              rdsb[0:Ka + 1, :])

        # ---------------- write outputs ----------------
        for b in range(B2):
            nY.dma_start(out=d_out[b], in_=means_acc[:, b, :])

    return split_waits(nc)


# ======================= host side =======================

def _f16(x):
    return np.asarray(x, np.float32).astype(np.float16)


def _bf16(x):
    return np.asarray(x, np.float32).astype(ml_dtypes.bfloat16)


def _rep_row(row, p=128):
    """replicate a row vector across 128 partitions."""
    r = np.asarray(row, np.float32).reshape(-1)
    return np.broadcast_to(r[None, :], (p, r.shape[0])).copy()


def host_prep(inputs, t_steps=T_FULL):
    obs = np.asarray(inputs["obs"], np.float32)[:t_steps]
    h_seq = np.asarray(inputs["h_seq"], np.float32)[:t_steps]
    z0 = np.asarray(inputs["z0"], np.float32)
    rl0 = np.asarray(inputs["regime_logits0"], np.float32)
    eps = np.asarray(inputs["eps"], np.float32)[:t_steps]
    u = np.asarray(inputs["gumbel_u"], np.float32)[:t_steps]
    assert int(inputs["k_active"]) == Ka

    pz_w1 = np.asarray(inputs["pz_w1"], np.float32)
    pr_w1 = np.asarray(inputs["pr_w1"], np.float32)
    oe_w1 = np.asarray(inputs["oe_w1"], np.float32)
    pz_b1 = np.asarray(inputs["pz_b1"], np.float32)
    pr_b1 = np.asarray(inputs["pr_b1"], np.float32)
    oe_b1 = np.asarray(inputs["oe_b1"], np.float32)
    pz_b3 = np.asarray(inputs["pz_b3"], np.float32)
    oe_b3 = np.asarray(inputs["oe_b3"], np.float32)
    emb_a = np.asarray(inputs["pe_emb"], np.float32)[:Ka]
    b3m = pz_b3[:dL]

    # exp(g/TEMP) = x^-2  with x = -log(u+1e-10)+1e-10 (TEMP=0.5)
    x = (-np.log(u + np.float32(1e-10)) + np.float32(1e-10)).astype(np.float32)
    P = (1.0 / (x * x)).astype(np.float32)

    # t=0 regime softmax (active slice), rows layout
    e0 = np.exp(rl0 - rl0.max(axis=-1, keepdims=True))
    rp0 = (e0 / e0.sum(axis=-1, keepdims=True))[:, :, :Ka]  # (B,N,8)

    # per-(t,b) layer-1 bias pack + y' + qeps
    # l1 input rows: [h (dM), z (dL), e (dE)]
    qeps_const = (dL - 1) * 0.5 * LOG2PI

    def l1_bias(w1, b1v):
        bias_t = np.einsum('tbm,mh->tbh', h_seq, w1[:dM]) + b1v
        bias_t[1:] += w1[dM:dM + dL].T @ b3m   # carry excludes b3m for t>=1
        return bias_t.reshape(t_steps, B, 2, 128)  # (t, B, m, 128)

    bias_pz = l1_bias(pz_w1, pz_b1)
    bias_pr = l1_bias(pr_w1, pr_b1)
    # oe bias: h part rows oe_w1[dL:], + oe1_z^T b3m always
    bias_oe = np.einsum('tbm,mh->tbh', h_seq, oe_w1[dL:]) + oe_b1
    bias_oe += oe_w1[:dL].T @ b3m
    bias_oe = bias_oe.reshape(t_steps, B, 2, 128)

    yprime = obs - oe_b3[0]                     # (t, B)
    qeps = 0.5 * (eps.astype(np.float32) ** 2).sum(-1) + qeps_const  # (t,B,N)

    wmap = {
        "pz1_z": _f16(pz_w1[dM:dM + dL]),
        "pr1_z": _f16(pr_w1[dM:dM + dL]),
        "oe1_z": _f16(oe_w1[:dL]),
        "vpz": _f16(emb_a @ pz_w1[dM + dL:]),
        "vpr": _f16(emb_a @ pr_w1[dM + dL:]),
        "pz2": _f16(inputs["pz_w2"]), "pz3": _f16(inputs["pz_w3"]),
        "pr2": _f16(inputs["pr_w2"]),
        "oe2": _f16(inputs["oe_w2"]), "oe3": _f16(inputs["oe_w3"]),
        "b2z": np.asarray(inputs["pz_b2"], np.float32).reshape(2, 128).T.copy(),
        "b2o": np.asarray(inputs["oe_b2"], np.float32).reshape(1, 128).T.copy(),
        "b3s_rep": np.tile(_rep_row(pz_b3[dL:]), (1, NC)),
        "b3m_col": np.repeat(b3m[:, None], 1, axis=1).astype(np.float32),
        "oe3b1": np.full((128, 1), oe_b3[1], np.float32),
        "scales_rep": np.tile(_rep_row(np.log1p(np.exp(
            np.asarray(inputs["log_obs_scale"], np.float32)[:Ka]))), (1, NCB)),
        "prb2_rep": np.tile(_rep_row(np.asarray(inputs["pr_b2"], np.float32)),
                            (1, NCB)),
    }

    in_maps = []
    for core in range(NCORES):
        bp = [2 * core, 2 * core + 1]
        m = dict(wmap)
        m["pT"] = _bf16(P[:, bp].transpose(0, 1, 3, 2))       # (T,2,j,i)
        m["eps_n"] = _f16(eps[:, bp])                          # (T,2,N,dL)
        m["z0T"] = _f16(np.concatenate([z0[b].T for b in bp], axis=1))
        m["rp0T"] = _f16(np.concatenate([rp0[b].T for b in bp], axis=1))
        hbc = np.zeros((t_steps, 128, NHB), np.float32)
        for bi, bb in enumerate(bp):
            for mm in range(2):
                hbc[:, :, 0 * 4 + mm * 2 + bi] = bias_pz[:, bb, mm]
                hbc[:, :, 1 * 4 + mm * 2 + bi] = bias_pr[:, bb, mm]
                hbc[:, :, 2 * 4 + mm * 2 + bi] = bias_oe[:, bb, mm]
            hbc[:, :, 12 + bi] = yprime[:, bb, None]
            q = qeps[:, bb].reshape(t_steps, NC, 128)          # (t, jc, j)
            for jc in range(NC):
                hbc[:, :, 14 + bi * NC + jc] = q[:, jc]
        m["hb"] = hbc
        in_maps.append(m)
    return in_maps


def gather_output(results, t_steps=T_FULL):
    out = np.zeros((t_steps, B, dL), np.float32)
    for core in range(NCORES):
        r = results[core]["means"]                             # (2,128,T)
        for b in range(B2):
            out[:, 2 * core + b, :] = np.asarray(r[b], np.float32).T
    return out


def kernel(**inputs):
    from concourse.bass_utils import run_bass_kernel_spmd
    nc = build_core_program(T_FULL)
    in_maps = host_prep(inputs, T_FULL)
    res = run_bass_kernel_spmd(nc, in_maps, list(range(NCORES)))
    return gather_output(res.results, T_FULL)


if __name__ == "__main__":
    nc = build_core_program(2)
    print("built ok")
